# revision 31
# baseline (speedup 1.0000x reference)
"""Trainium2 Bass kernel for the attention-pooling module.

Reference math (B=32, N=2048, D=512, K=256):
    vIp   = vI @ Wi                                   [B,N,K]
    vQp   = vQ @ Wq + bq                              [B,K]
    ha    = leaky_relu(vIp + vQp[:,None,:], 0.01)     [B,N,K]
    scores= ha @ Wp[:,0] + bp                         [B,N]   (bp shift cancels in softmax)
    pi    = softmax(scores, -1)                       [B,N]
    out   = einsum("bn,bnk->bk", pi, vIp) + vQp       [B,K]

Kernel strategy (8 cores, data-parallel over B, 4 batches/core):
  - The output is vQp-dominated: vI_attn is a pi-weighted mean of ~N(0,0.58)
    rows over 2048 samples, ~40x smaller than vQp. Errors in the whole
    scores/attention path are damped accordingly, so vI streams as fp8-e4m3
    (host-cast), 1 MiB per batch; the vQp path stays fp32.
  - vI is host-transposed to [D, N] so the device streams vIT at natural-DMA
    rate and nothing on-chip transposes the bulk tensor (PE-mode transposes
    cost ~275 ns each and starve the HAM clock gate).
  - vIpT = Wi.T @ vIT in [K-on-partitions, N-on-free] layout (fp8 x fp8
    matmuls), so the vQp bias, Wp weighting and softmax map onto
    per-partition ops.
  - ha = ACT Lrelu(vIpT + vQp_k) fused, emitted as [128,1024] double-wides
    to halve ACT instruction count.
  - scores = matmul(lhsT=Wp_col, rhs=ha) accumulated over the two K chunks;
    exp without max-subtraction (|scores| < ~2) with Z via accum_out.
  - u = e @ vI on DVE via the fused affine_mul_reduce custom op against a
    GpSimd partition_broadcast of e (single pass over vIT per batch).
  - vI_attn = (u @ Wi) / Z  (exact linear refactor of pi @ vIp).
  - The scores phase (PE-bound) of batch b+1 is interleaved with the
    attention phase (DVE-bound) of batch b so the two bottleneck engines
    overlap.
"""

import os
import sys

sys.path.insert(0, "/opt/trn_rl_repo")

import numpy as np
import ml_dtypes

from concourse import bass, bacc, tile, mybir
from concourse.bass_utils import run_bass_kernel_spmd

dt = mybir.dt
F32, BF16, FP8 = dt.float32, dt.bfloat16, dt.float8e4
AF = mybir.ActivationFunctionType
ALU = mybir.AluOpType
AXF = mybir.AxisListType.X

B, N, D, K = 32, 2048, 512, 256
NCORES = 8
BLOC = B // NCORES           # 4 batches per core
SUP = 512                    # scores supertile (PSUM-bank limited)
NSUP = N // SUP              # 4
WSUP = 1024                  # ha double-wide
DC = D // 128                # 4 contraction chunks
KC = K // 128                # 2 K chunks
NEG = 0.01


def build_nc():
    nc = bacc.Bacc("TRN2", target_bir_lowering=False, debug=False)

    vit_d = nc.dram_tensor("vit", [BLOC, DC, 128, N], FP8, kind="ExternalInput")
    vq = nc.dram_tensor("vq", [BLOC, D], F32, kind="ExternalInput")
    wi8 = nc.dram_tensor("wi8", [128, DC, K], FP8, kind="ExternalInput")
    wib = nc.dram_tensor("wib", [128, DC, K], BF16, kind="ExternalInput")
    wq = nc.dram_tensor("wq", [128, DC, K], F32, kind="ExternalInput")
    bqc = nc.dram_tensor("bqc", [128, KC], F32, kind="ExternalInput")
    wpc = nc.dram_tensor("wpc", [128, KC], BF16, kind="ExternalInput")
    idf = nc.dram_tensor("idf", [128, 128], F32, kind="ExternalInput")
    out = nc.dram_tensor("out", [BLOC, K], F32, kind="ExternalOutput")
    DEBUG = bool(int(os.environ.get("KERNEL_DEBUG", "0")))
    DBG_B = int(os.environ.get("KERNEL_DEBUG_B", "0"))
    if DEBUG:
        d_erow = nc.dram_tensor("d_erow", [1, N], FP8, kind="ExternalOutput")
        d_z = nc.dram_tensor("d_z", [1, 1], F32, kind="ExternalOutput")
        d_ucol = nc.dram_tensor("d_ucol", [128, DC], F32, kind="ExternalOutput")
        d_fin = nc.dram_tensor("d_fin", [1, K], F32, kind="ExternalOutput")

    with tile.TileContext(nc) as tc:
        with (
            tc.tile_pool(name="const", bufs=1) as cpool,
            tc.tile_pool(name="stream", bufs=4) as spool,
            tc.tile_pool(name="work", bufs=3) as wpool,
            tc.tile_pool(name="pmm", bufs=3, space=bass.MemorySpace.PSUM) as pmm,
            tc.tile_pool(name="psm", bufs=2, space=bass.MemorySpace.PSUM) as psm,
        ):
            # ---- prefetch the big streams first (latency-critical) ----
            vit_tiles = []
            for b in range(BLOC):
                vit = spool.tile([128, DC, N], FP8, tag="vit", name=f"vit{b}")
                src = vit_d[b].rearrange("c p n -> p c n")
                nc.sync.dma_start(out=vit[:, :, 0:1024], in_=src[:, :, 0:1024])
                nc.sync.dma_start(out=vit[:, :, 1024:N], in_=src[:, :, 1024:N])
                vit_tiles.append(vit)

            # ---- constants / weights ----
            wi8_sb = cpool.tile([128, DC, K], FP8, tag="wi8")
            wib_sb = cpool.tile([128, DC, K], BF16, tag="wib")
            wq_sb = cpool.tile([128, DC, K], F32, tag="wq")
            bq_sb = cpool.tile([128, KC], F32, tag="bq")
            wp_sb = cpool.tile([128, KC], BF16, tag="wp")
            idf_sb = cpool.tile([128, 128], F32, tag="idf")
            nc.sync.dma_start(out=wi8_sb[:], in_=wi8[:])
            nc.sync.dma_start(out=wib_sb[:], in_=wib[:])
            nc.sync.dma_start(out=wq_sb[:], in_=wq[:])
            nc.sync.dma_start(out=bq_sb[:], in_=bqc[:])
            nc.sync.dma_start(out=wp_sb[:], in_=wpc[:])
            nc.sync.dma_start(out=idf_sb[:], in_=idf[:])

            # ---- vQp (fp32, once per core, all 4 local batches) ----
            vq_sb = cpool.tile([BLOC, D], F32, tag="vqsb")
            nc.sync.dma_start(out=vq_sb[:], in_=vq[:])

            vqt_ps = psm.tile([128, DC, BLOC], F32, tag="small")
            for c in range(DC):
                nc.tensor.transpose(
                    vqt_ps[:, c, :],
                    vq_sb[:, c * 128 : (c + 1) * 128],
                    idf_sb[0:BLOC, 0:BLOC],
                )
            vqt_sb = cpool.tile([128, DC, BLOC], F32, tag="vqt")
            nc.vector.tensor_copy(vqt_sb[:], vqt_ps[:])

            # vQp^T[k, b] = sum_d Wq[d,k] vQ[b,d] + bq[k]   (K on partitions)
            vqpt_sb = cpool.tile([128, KC, BLOC], F32, tag="vqpt")
            for kc in range(KC):
                vqpt_ps = psm.tile([128, BLOC], F32, tag="small")
                for c in range(DC):
                    nc.tensor.matmul(
                        vqpt_ps[:],
                        wq_sb[:, c, kc * 128 : (kc + 1) * 128],
                        vqt_sb[:, c, :],
                        start=(c == 0),
                        stop=(c == DC - 1),
                    )
                nc.vector.tensor_scalar(
                    vqpt_sb[:, kc, :], vqpt_ps[:], bq_sb[:, kc : kc + 1], None, ALU.add
                )

            # row form vQp[b] = [1, K]  (transpose back; includes bq)
            vqpr_sb = cpool.tile([1, BLOC, K], F32, tag="vqpr")
            for b in range(BLOC):
                vqpr_ps = psm.tile([1, K], F32, tag="small")
                for kc in range(KC):
                    nc.tensor.transpose(
                        vqpr_ps[0:1, kc * 128 : (kc + 1) * 128],
                        vqpt_sb[:, kc, b : b + 1],
                        idf_sb[:],
                    )
                nc.vector.tensor_copy(vqpr_sb[:, b, :], vqpr_ps[:])

            out_sb = cpool.tile([1, BLOC, K], F32, tag="outb")

            vits, scrows = [None] * BLOC, [None] * BLOC

            def phase_scores(b):
                vit = vit_tiles[b]
                vits[b] = vit
                scrow = wpool.tile([1, N], F32, tag="scrow")
                scrows[b] = scrow
                for sp in range(N // WSUP):           # two 1024-wide supertiles
                    scps = [
                        psm.tile([1, SUP], F32, tag="small", name=f"scp{b}_{sp}_{h}")
                        for h in range(2)
                    ]
                    has = []
                    for kc in range(KC):
                        vp = pmm.tile([128, WSUP], F32, tag="vp")
                        for h in range(2):
                            n0 = sp * WSUP + h * SUP
                            for c in range(DC):
                                nc.tensor.matmul(
                                    vp[:, h * SUP : (h + 1) * SUP],
                                    wi8_sb[:, c, kc * 128 : (kc + 1) * 128],
                                    vit[:, c, n0 : n0 + SUP],
                                    start=(c == 0),
                                    stop=(c == DC - 1),
                                )
                        ha = wpool.tile([128, WSUP], BF16, tag="ha")
                        # Wi is host-scaled x16 into fp8 normal range; ACT
                        # de-scales for free: ha = lrelu(vp/16 + vqp)
                        nc.scalar.activation(
                            ha[:], vp[:], AF.Lrelu,
                            bias=vqpt_sb[:, kc, b : b + 1], scale=1.0 / 16, alpha=NEG,
                        )
                        has.append(ha)
                    for kc in range(KC):
                        for h in range(2):
                            nc.tensor.matmul(
                                scps[h][:], wp_sb[:, kc : kc + 1],
                                has[kc][:, h * SUP : (h + 1) * SUP],
                                start=(kc == 0), stop=(kc == KC - 1),
                            )
                    for h in range(2):
                        n0 = sp * WSUP + h * SUP
                        if h == 0:
                            nc.scalar.copy(scrow[0:1, n0 : n0 + SUP], scps[h][:])
                        else:
                            nc.vector.tensor_copy(scrow[0:1, n0 : n0 + SUP], scps[h][:])

            def phase_attn(b):
                vit, scrow = vits[b], scrows[b]
                e_row = wpool.tile([1, N], FP8, tag="erow")
                z_sb = wpool.tile([1, 1], F32, tag="zsb")
                nc.scalar.activation(e_row[:], scrow[:], AF.Exp, accum_out=z_sb[:])
                invz = wpool.tile([1, 1], F32, tag="invz")
                nc.vector.reciprocal(invz[:], z_sb[:])

                # broadcast e to all partitions (GpSimd custom op, idle engine)
                e_bc = wpool.tile([128, N], FP8, tag="ebc")
                nc.gpsimd.partition_broadcast(e_bc[:], e_row[0:1, :])

                # u[d] = sum_n e[n] vIT[d, n]  -- fused DVE multiply+reduce
                ucol = wpool.tile([128, DC], F32, tag="ucol")
                uscr = wpool.tile([128, N], BF16, tag="uscr")
                for c in range(DC):
                    nc.vector.affine_mul_reduce(
                        uscr[:], ucol[:, c : c + 1], vit[:, c, :], e_bc[:], 1.0, 0.0
                    )
                ucb = wpool.tile([128, DC], BF16, tag="ucb")
                nc.vector.tensor_copy(ucb[:], ucol[:])

                # att = u @ Wi   [1, K]
                atp = psm.tile([1, K], F32, tag="small")
                for c in range(DC):
                    nc.tensor.matmul(
                        atp[:], ucb[:, c : c + 1], wib_sb[:, c, :],
                        start=(c == 0), stop=(c == DC - 1),
                    )
                fin = wpool.tile([1, K], F32, tag="fin")
                nc.vector.tensor_scalar(fin[:], atp[:], invz[:], None, ALU.mult)
                nc.vector.tensor_tensor(
                    out_sb[:, b, :], fin[:], vqpr_sb[:, b, :], ALU.add
                )
                if DEBUG and b == DBG_B:
                    nc.sync.dma_start(out=d_erow[:], in_=e_row[:])
                    nc.sync.dma_start(out=d_z[:], in_=z_sb[:])
                    nc.sync.dma_start(out=d_ucol[:], in_=ucol[:])
                    nc.sync.dma_start(out=d_fin[:], in_=fin[:])

            # software pipeline: scores(b+1) overlaps attention(b)
            for b in range(BLOC + 1):
                if b < BLOC:
                    phase_scores(b)
                if b >= 1:
                    phase_attn(b - 1)

            nc.sync.dma_start(out=out[:, :], in_=out_sb[0:1, :, :])

    nc.compile()
    return nc


_NC = None


def _get_nc():
    global _NC
    if _NC is None:
        _NC = build_nc()
    return _NC


def kernel(vI, vQ, Wi, Wq, bq, Wp, bp, **_unused):
    vI = np.asarray(vI, dtype=np.float32)
    vQ = np.asarray(vQ, dtype=np.float32)
    Wi = np.asarray(Wi, dtype=np.float32)
    Wq = np.asarray(Wq, dtype=np.float32)
    bq = np.asarray(bq, dtype=np.float32)
    Wp = np.asarray(Wp, dtype=np.float32)
    # bp shifts every score equally -> cancels in softmax; ignored.

    bf = ml_dtypes.bfloat16
    f8 = ml_dtypes.float8_e4m3
    # host-side: cast to fp8 and pre-transpose to [B, DC, 128, N]
    viT = np.ascontiguousarray(
        vI.astype(f8).transpose(0, 2, 1).reshape(B, DC, 128, N)
    )
    wi_r = Wi.reshape(DC, 128, K).transpose(1, 0, 2)             # [128,DC,K]
    wq_h = np.ascontiguousarray(Wq.reshape(DC, 128, K).transpose(1, 0, 2))
    bq_h = np.ascontiguousarray(bq.reshape(KC, 128).T)           # [128,KC]
    wp_h = np.ascontiguousarray(Wp[:, 0].reshape(KC, 128).T).astype(bf)
    idf = np.eye(128, dtype=np.float32)

    in_maps = []
    for c in range(NCORES):
        in_maps.append(
            {
                "vit": viT[c * BLOC : (c + 1) * BLOC],
                "vq": np.ascontiguousarray(vQ[c * BLOC : (c + 1) * BLOC]),
                "wi8": (wi_r * 16.0).astype(f8),
                "wib": wi_r.astype(bf),
                "wq": wq_h,
                "bqc": bq_h,
                "wpc": wp_h,
                "idf": idf,
            }
        )

    nc = _get_nc()
    res = run_bass_kernel_spmd(
        nc, in_maps, list(range(NCORES)),
        trace=bool(int(os.environ.get("KERNEL_TRACE", "0"))),
        tmpdir=globals().get("TRACE_TMPDIR"),
    )
    kernel.last_results = res
    return np.concatenate([res.results[c]["out"] for c in range(NCORES)], axis=0)


# revision 34
# speedup vs baseline: 1.0178x; 1.0178x over previous
"""Trainium2 Bass kernel for the attention-pooling module.

Reference math (B=32, N=2048, D=512, K=256):
    vIp   = vI @ Wi                                   [B,N,K]
    vQp   = vQ @ Wq + bq                              [B,K]
    ha    = leaky_relu(vIp + vQp[:,None,:], 0.01)     [B,N,K]
    scores= ha @ Wp[:,0] + bp                         [B,N]   (bp shift cancels in softmax)
    pi    = softmax(scores, -1)                       [B,N]
    out   = einsum("bn,bnk->bk", pi, vIp) + vQp       [B,K]

Kernel strategy (8 cores, data-parallel over B, 4 batches/core):
  - The output is vQp-dominated: vI_attn is a pi-weighted mean of ~N(0,0.58)
    rows over 2048 samples, ~40x smaller than vQp. Errors in the whole
    scores/attention path are damped accordingly, so vI streams as fp8-e4m3
    (host-cast), 1 MiB per batch; the vQp path stays fp32.
  - vI is host-transposed to [D, N] so the device streams vIT at natural-DMA
    rate and nothing on-chip transposes the bulk tensor (PE-mode transposes
    cost ~275 ns each and starve the HAM clock gate).
  - vIpT = Wi.T @ vIT in [K-on-partitions, N-on-free] layout (fp8 x fp8
    matmuls), so the vQp bias, Wp weighting and softmax map onto
    per-partition ops.
  - ha = ACT Lrelu(vIpT + vQp_k) fused, emitted as [128,1024] double-wides
    to halve ACT instruction count.
  - scores = matmul(lhsT=Wp_col, rhs=ha) accumulated over the two K chunks;
    exp without max-subtraction (|scores| < ~2) with Z via accum_out.
  - u = e @ vI on DVE via the fused affine_mul_reduce custom op against a
    GpSimd partition_broadcast of e (single pass over vIT per batch).
  - vI_attn = (u @ Wi) / Z  (exact linear refactor of pi @ vIp).
  - The scores phase (PE-bound) of batch b+1 is interleaved with the
    attention phase (DVE-bound) of batch b so the two bottleneck engines
    overlap.
"""

import os
import sys

sys.path.insert(0, "/opt/trn_rl_repo")

import numpy as np
import ml_dtypes

from concourse import bass, bacc, tile, mybir
from concourse.bass_utils import run_bass_kernel_spmd

dt = mybir.dt
F32, BF16, FP8 = dt.float32, dt.bfloat16, dt.float8e4
AF = mybir.ActivationFunctionType
ALU = mybir.AluOpType
AXF = mybir.AxisListType.X

B, N, D, K = 32, 2048, 512, 256
NCORES = 8
BLOC = B // NCORES           # 4 batches per core
SUP = 512                    # scores supertile (PSUM-bank limited)
NSUP = N // SUP              # 4
WSUP = 1024                  # ha double-wide
DC = D // 128                # 4 contraction chunks
KC = K // 128                # 2 K chunks
NEG = 0.01


def build_nc():
    nc = bacc.Bacc("TRN2", target_bir_lowering=False, debug=False)

    vit_d = nc.dram_tensor("vit", [BLOC, DC, 128, N], FP8, kind="ExternalInput")
    vnat_d = nc.dram_tensor("vnat", [BLOC, 128, N // 128, D], FP8, kind="ExternalInput")
    vq = nc.dram_tensor("vq", [BLOC, D], F32, kind="ExternalInput")
    wi8 = nc.dram_tensor("wi8", [128, DC, K], FP8, kind="ExternalInput")
    wib = nc.dram_tensor("wib", [128, DC, K], BF16, kind="ExternalInput")
    wq = nc.dram_tensor("wq", [128, DC, K], F32, kind="ExternalInput")
    bqc = nc.dram_tensor("bqc", [128, KC], F32, kind="ExternalInput")
    wpc = nc.dram_tensor("wpc", [128, KC], BF16, kind="ExternalInput")
    idf = nc.dram_tensor("idf", [128, 128], F32, kind="ExternalInput")
    idb = nc.dram_tensor("idb", [128, 128], BF16, kind="ExternalInput")
    onesc = nc.dram_tensor("onesc", [128, 1], F32, kind="ExternalInput")
    out = nc.dram_tensor("out", [BLOC, K], F32, kind="ExternalOutput")
    DEBUG = bool(int(os.environ.get("KERNEL_DEBUG", "0")))
    DBG_B = int(os.environ.get("KERNEL_DEBUG_B", "0"))
    if DEBUG:
        d_ecol = nc.dram_tensor("d_ecol", [128, 16], FP8, kind="ExternalOutput")
        d_z = nc.dram_tensor("d_z", [1, 1], F32, kind="ExternalOutput")
        d_fin = nc.dram_tensor("d_fin", [1, K], F32, kind="ExternalOutput")

    with tile.TileContext(nc) as tc:
        with (
            tc.tile_pool(name="const", bufs=1) as cpool,
            tc.tile_pool(name="stream", bufs=4) as spool,
            tc.tile_pool(name="work", bufs=3) as wpool,
            tc.tile_pool(name="pmm", bufs=3, space=bass.MemorySpace.PSUM) as pmm,
            tc.tile_pool(name="psm", bufs=2, space=bass.MemorySpace.PSUM) as psm,
        ):
            # ---- prefetch the big streams first (latency-critical) ----
            vit_tiles, vnat_tiles = [], []
            for b in range(BLOC):
                vit = spool.tile([128, DC, N], FP8, tag="vit", name=f"vit{b}")
                vsrc = vit_d[b].rearrange("c p n -> p c n")
                nc.sync.dma_start(out=vit[:, :, 0:1024], in_=vsrc[:, :, 0:1024])
                nc.sync.dma_start(out=vit[:, :, 1024:N], in_=vsrc[:, :, 1024:N])
                vit_tiles.append(vit)
                vnat = spool.tile([128, N // 128, D], FP8, tag="vnat", name=f"vnat{b}")
                nc.sync.dma_start(out=vnat[:], in_=vnat_d[b])
                vnat_tiles.append(vnat)

            # ---- constants / weights ----
            wi8_sb = cpool.tile([128, DC, K], FP8, tag="wi8")
            wib_sb = cpool.tile([128, DC, K], BF16, tag="wib")
            wq_sb = cpool.tile([128, DC, K], F32, tag="wq")
            bq_sb = cpool.tile([128, KC], F32, tag="bq")
            wp_sb = cpool.tile([128, KC], BF16, tag="wp")
            idf_sb = cpool.tile([128, 128], F32, tag="idf")
            nc.sync.dma_start(out=wi8_sb[:], in_=wi8[:])
            nc.sync.dma_start(out=wib_sb[:], in_=wib[:])
            nc.sync.dma_start(out=wq_sb[:], in_=wq[:])
            nc.sync.dma_start(out=bq_sb[:], in_=bqc[:])
            nc.sync.dma_start(out=wp_sb[:], in_=wpc[:])
            nc.sync.dma_start(out=idf_sb[:], in_=idf[:])
            idb_sb = cpool.tile([128, 128], BF16, tag="idb")
            onesc_sb = cpool.tile([128, 1], F32, tag="onesc")
            nc.sync.dma_start(out=idb_sb[:], in_=idb[:])
            nc.sync.dma_start(out=onesc_sb[:], in_=onesc[:])

            # ---- vQp (fp32, once per core, all 4 local batches) ----
            vq_sb = cpool.tile([BLOC, D], F32, tag="vqsb")
            nc.sync.dma_start(out=vq_sb[:], in_=vq[:])

            vqt_ps = psm.tile([128, DC, BLOC], F32, tag="small")
            for c in range(DC):
                nc.tensor.transpose(
                    vqt_ps[:, c, :],
                    vq_sb[:, c * 128 : (c + 1) * 128],
                    idf_sb[0:BLOC, 0:BLOC],
                )
            vqt_sb = cpool.tile([128, DC, BLOC], F32, tag="vqt")
            nc.vector.tensor_copy(vqt_sb[:], vqt_ps[:])

            # vQp^T[k, b] = sum_d Wq[d,k] vQ[b,d] + bq[k]   (K on partitions)
            vqpt_sb = cpool.tile([128, KC, BLOC], F32, tag="vqpt")
            for kc in range(KC):
                vqpt_ps = psm.tile([128, BLOC], F32, tag="small")
                for c in range(DC):
                    nc.tensor.matmul(
                        vqpt_ps[:],
                        wq_sb[:, c, kc * 128 : (kc + 1) * 128],
                        vqt_sb[:, c, :],
                        start=(c == 0),
                        stop=(c == DC - 1),
                    )
                nc.vector.tensor_scalar(
                    vqpt_sb[:, kc, :], vqpt_ps[:], bq_sb[:, kc : kc + 1], None, ALU.add
                )

            # row form vQp[b] = [1, K]  (transpose back; includes bq)
            vqpr_sb = cpool.tile([1, BLOC, K], F32, tag="vqpr")
            for b in range(BLOC):
                vqpr_ps = psm.tile([1, K], F32, tag="small")
                for kc in range(KC):
                    nc.tensor.transpose(
                        vqpr_ps[0:1, kc * 128 : (kc + 1) * 128],
                        vqpt_sb[:, kc, b : b + 1],
                        idf_sb[:],
                    )
                nc.vector.tensor_copy(vqpr_sb[:, b, :], vqpr_ps[:])

            out_sb = cpool.tile([1, BLOC, K], F32, tag="outb")

            vits, scrows = [None] * BLOC, [None] * BLOC

            def phase_scores(b):
                vit = vit_tiles[b]
                vits[b] = vit
                scrow = wpool.tile([1, N], F32, tag="scrow")
                scrows[b] = scrow
                for sp in range(N // WSUP):           # two 1024-wide supertiles
                    scps = [
                        psm.tile([1, SUP], F32, tag="small", name=f"scp{b}_{sp}_{h}")
                        for h in range(2)
                    ]
                    has = []
                    for kc in range(KC):
                        vp = pmm.tile([128, WSUP], F32, tag="vp")
                        for h in range(2):
                            n0 = sp * WSUP + h * SUP
                            for c in range(DC):
                                nc.tensor.matmul(
                                    vp[:, h * SUP : (h + 1) * SUP],
                                    wi8_sb[:, c, kc * 128 : (kc + 1) * 128],
                                    vit[:, c, n0 : n0 + SUP],
                                    start=(c == 0),
                                    stop=(c == DC - 1),
                                )
                        ha = wpool.tile([128, WSUP], BF16, tag="ha")
                        # Wi is host-scaled x16 into fp8 normal range; ACT
                        # de-scales for free: ha = lrelu(vp/16 + vqp)
                        nc.scalar.activation(
                            ha[:], vp[:], AF.Lrelu,
                            bias=vqpt_sb[:, kc, b : b + 1], scale=1.0 / 16, alpha=NEG,
                        )
                        has.append(ha)
                    for kc in range(KC):
                        for h in range(2):
                            nc.tensor.matmul(
                                scps[h][:], wp_sb[:, kc : kc + 1],
                                has[kc][:, h * SUP : (h + 1) * SUP],
                                start=(kc == 0), stop=(kc == KC - 1),
                            )
                    for h in range(2):
                        n0 = sp * WSUP + h * SUP
                        if h == 0:
                            nc.scalar.copy(scrow[0:1, n0 : n0 + SUP], scps[h][:])
                        else:
                            nc.vector.tensor_copy(scrow[0:1, n0 : n0 + SUP], scps[h][:])

            def phase_attn(b):
                vit, vnat, scrow = vits[b], vnat_tiles[b], scrows[b]
                # scores -> [16,128] -> xbar transpose -> [128,16] (partition-
                # major), then exp there. Both hops are tiny SBUF->SBUF DMAs
                # on otherwise-idle queues.
                s16 = wpool.tile([16, 128], BF16, tag="s16")
                sb16 = wpool.tile([1, N], BF16, tag="sb16")
                nc.vector.tensor_copy(sb16[:], scrow[:])
                nc.sync.dma_start(
                    out=s16[:], in_=sb16[0:1, :].rearrange("o (t p) -> o t p", p=128)
                )
                s_col = wpool.tile([128, 16], BF16, tag="scol")
                nc.sync.dma_start_transpose(out=s_col[:], in_=s16[:])

                e_col = wpool.tile([128, 16], FP8, tag="ecol")
                zp = wpool.tile([128, 1], F32, tag="zp")
                nc.scalar.activation(e_col[:], s_col[:], AF.Exp, accum_out=zp[:])
                zps = psm.tile([1, 1], F32, tag="small")
                nc.tensor.matmul(zps[:], onesc_sb[:], zp[:], start=True, stop=True)
                z_sb = wpool.tile([1, 1], F32, tag="zsb")
                nc.vector.tensor_copy(z_sb[:], zps[:])
                invz = wpool.tile([1, 1], F32, tag="invz")
                nc.vector.reciprocal(invz[:], z_sb[:])

                # u = e @ vI on the PE: 16 accumulating fp8 matmuls
                ups = psm.tile([1, D], F32, tag="small")
                for t in range(N // 128):
                    nc.tensor.matmul(
                        ups[:],
                        e_col[:, t : t + 1],
                        vnat[:, t, :],
                        start=(t == 0),
                        stop=(t == N // 128 - 1),
                    )
                u_sb = wpool.tile([1, D], BF16, tag="usb")
                nc.vector.tensor_copy(u_sb[:], ups[:])
                utp = psm.tile([128, DC, 2], BF16, tag="small")
                for c in range(DC):
                    nc.tensor.transpose(
                        utp[:, c, 0:1],
                        u_sb[0:1, c * 128 : (c + 1) * 128],
                        idb_sb[0:1, 0:1],
                    )
                ut_sb = wpool.tile([128, DC], BF16, tag="utsb")
                nc.vector.tensor_copy(ut_sb[:], utp[:, :, 0])

                # att = u @ Wi   [1, K]
                atp = psm.tile([1, K], F32, tag="small")
                for c in range(DC):
                    nc.tensor.matmul(
                        atp[:], ut_sb[:, c : c + 1], wib_sb[:, c, :],
                        start=(c == 0), stop=(c == DC - 1),
                    )
                fin = wpool.tile([1, K], F32, tag="fin")
                nc.vector.tensor_scalar(fin[:], atp[:], invz[:], None, ALU.mult)
                nc.vector.tensor_tensor(
                    out_sb[:, b, :], fin[:], vqpr_sb[:, b, :], ALU.add
                )
                if DEBUG and b == DBG_B:
                    nc.sync.dma_start(out=d_ecol[:], in_=e_col[:])
                    nc.sync.dma_start(out=d_z[:], in_=z_sb[:])
                    nc.sync.dma_start(out=d_fin[:], in_=fin[:])

            # software pipeline: scores(b+1) overlaps attention(b)
            for b in range(BLOC + 1):
                if b < BLOC:
                    phase_scores(b)
                if b >= 1:
                    phase_attn(b - 1)

            nc.sync.dma_start(out=out[:, :], in_=out_sb[0:1, :, :])

    nc.compile()
    return nc


_NC = None


def _get_nc():
    global _NC
    if _NC is None:
        _NC = build_nc()
    return _NC


def kernel(vI, vQ, Wi, Wq, bq, Wp, bp, **_unused):
    vI = np.asarray(vI, dtype=np.float32)
    vQ = np.asarray(vQ, dtype=np.float32)
    Wi = np.asarray(Wi, dtype=np.float32)
    Wq = np.asarray(Wq, dtype=np.float32)
    bq = np.asarray(bq, dtype=np.float32)
    Wp = np.asarray(Wp, dtype=np.float32)
    # bp shifts every score equally -> cancels in softmax; ignored.

    bf = ml_dtypes.bfloat16
    f8 = ml_dtypes.float8_e4m3
    # host-side: cast to fp8 and pre-transpose to [B, DC, 128, N]
    vi8 = vI.astype(f8)
    viT = np.ascontiguousarray(vi8.transpose(0, 2, 1).reshape(B, DC, 128, N))
    vnat = np.ascontiguousarray(
        vi8.reshape(B, N // 128, 128, D).transpose(0, 2, 1, 3)
    )
    wi_r = Wi.reshape(DC, 128, K).transpose(1, 0, 2)             # [128,DC,K]
    wq_h = np.ascontiguousarray(Wq.reshape(DC, 128, K).transpose(1, 0, 2))
    bq_h = np.ascontiguousarray(bq.reshape(KC, 128).T)           # [128,KC]
    wp_h = np.ascontiguousarray(Wp[:, 0].reshape(KC, 128).T).astype(bf)
    idf = np.eye(128, dtype=np.float32)
    idb = idf.astype(bf)
    onesc = np.ones((128, 1), dtype=np.float32)

    in_maps = []
    for c in range(NCORES):
        in_maps.append(
            {
                "vit": viT[c * BLOC : (c + 1) * BLOC],
                "vnat": vnat[c * BLOC : (c + 1) * BLOC],
                "vq": np.ascontiguousarray(vQ[c * BLOC : (c + 1) * BLOC]),
                "wi8": (wi_r * 16.0).astype(f8),
                "wib": wi_r.astype(bf),
                "wq": wq_h,
                "bqc": bq_h,
                "wpc": wp_h,
                "idf": idf,
                "idb": idb,
                "onesc": onesc,
            }
        )

    nc = _get_nc()
    res = run_bass_kernel_spmd(
        nc, in_maps, list(range(NCORES)),
        trace=bool(int(os.environ.get("KERNEL_TRACE", "0"))),
        tmpdir=globals().get("TRACE_TMPDIR"),
    )
    kernel.last_results = res
    return np.concatenate([res.results[c]["out"] for c in range(NCORES)], axis=0)


# revision 35
# speedup vs baseline: 1.1290x; 1.1092x over previous
"""Trainium2 Bass kernel for the attention-pooling module.

Reference math (B=32, N=2048, D=512, K=256):
    vIp   = vI @ Wi                                   [B,N,K]
    vQp   = vQ @ Wq + bq                              [B,K]
    ha    = leaky_relu(vIp + vQp[:,None,:], 0.01)     [B,N,K]
    scores= ha @ Wp[:,0] + bp                         [B,N]   (bp shift cancels in softmax)
    pi    = softmax(scores, -1)                       [B,N]
    out   = einsum("bn,bnk->bk", pi, vIp) + vQp       [B,K]

Kernel strategy (8 cores, data-parallel over B, 4 batches/core):
  - The output is vQp-dominated: vI_attn is a pi-weighted mean of ~N(0,0.58)
    rows over 2048 samples, ~40x smaller than vQp. Errors in the whole
    scores/attention path are damped accordingly, so vI streams as fp8-e4m3
    (host-cast), 1 MiB per batch; the vQp path stays fp32.
  - vI is host-transposed to [D, N] so the device streams vIT at natural-DMA
    rate and nothing on-chip transposes the bulk tensor (PE-mode transposes
    cost ~275 ns each and starve the HAM clock gate).
  - vIpT = Wi.T @ vIT in [K-on-partitions, N-on-free] layout (fp8 x fp8
    matmuls), so the vQp bias, Wp weighting and softmax map onto
    per-partition ops.
  - ha = ACT Lrelu(vIpT + vQp_k) fused, emitted as [128,1024] double-wides
    to halve ACT instruction count.
  - scores = matmul(lhsT=Wp_col, rhs=ha) accumulated over the two K chunks;
    exp without max-subtraction (|scores| < ~2) with Z via accum_out.
  - u = e @ vI on DVE via the fused affine_mul_reduce custom op against a
    GpSimd partition_broadcast of e (single pass over vIT per batch).
  - vI_attn = (u @ Wi) / Z  (exact linear refactor of pi @ vIp).
  - The scores phase (PE-bound) of batch b+1 is interleaved with the
    attention phase (DVE-bound) of batch b so the two bottleneck engines
    overlap.
"""

import os
import sys

sys.path.insert(0, "/opt/trn_rl_repo")

import numpy as np
import ml_dtypes

from concourse import bass, bacc, tile, mybir
from concourse.bass_utils import run_bass_kernel_spmd

dt = mybir.dt
F32, BF16, FP8 = dt.float32, dt.bfloat16, dt.float8e4
AF = mybir.ActivationFunctionType
ALU = mybir.AluOpType
AXF = mybir.AxisListType.X

B, N, D, K = 32, 2048, 512, 256
NCORES = 8
BLOC = B // NCORES           # 4 batches per core
SUP = 512                    # scores supertile (PSUM-bank limited)
NSUP = N // SUP              # 4
WSUP = 1024                  # ha double-wide
DC = D // 128                # 4 contraction chunks
KC = K // 128                # 2 K chunks
NEG = 0.01


def build_nc():
    nc = bacc.Bacc("TRN2", target_bir_lowering=False, debug=False)

    vit_d = nc.dram_tensor("vit", [BLOC, DC, 128, N], FP8, kind="ExternalInput")
    vnat_d = nc.dram_tensor("vnat", [BLOC, 128, N // 128, D], FP8, kind="ExternalInput")
    vq = nc.dram_tensor("vq", [BLOC, D], F32, kind="ExternalInput")
    wi8 = nc.dram_tensor("wi8", [128, DC, K], FP8, kind="ExternalInput")
    wib = nc.dram_tensor("wib", [128, DC, K], BF16, kind="ExternalInput")
    wq = nc.dram_tensor("wq", [128, DC, K], F32, kind="ExternalInput")
    bqc = nc.dram_tensor("bqc", [128, KC], F32, kind="ExternalInput")
    wpc = nc.dram_tensor("wpc", [128, KC], BF16, kind="ExternalInput")
    idf = nc.dram_tensor("idf", [128, 128], F32, kind="ExternalInput")
    idb = nc.dram_tensor("idb", [128, 128], BF16, kind="ExternalInput")
    onesc = nc.dram_tensor("onesc", [128, 1], F32, kind="ExternalInput")
    out = nc.dram_tensor("out", [BLOC, K], F32, kind="ExternalOutput")
    DEBUG = bool(int(os.environ.get("KERNEL_DEBUG", "0")))
    DBG_B = int(os.environ.get("KERNEL_DEBUG_B", "0"))
    if DEBUG:
        d_ecol = nc.dram_tensor("d_ecol", [128, 16], FP8, kind="ExternalOutput")
        d_z = nc.dram_tensor("d_z", [1, 1], F32, kind="ExternalOutput")
        d_fin = nc.dram_tensor("d_fin", [1, K], F32, kind="ExternalOutput")

    with tile.TileContext(nc) as tc:
        with (
            tc.tile_pool(name="const", bufs=1) as cpool,
            tc.tile_pool(name="stream", bufs=4) as spool,
            tc.tile_pool(name="work", bufs=3) as wpool,
            tc.tile_pool(name="pmm", bufs=3, space=bass.MemorySpace.PSUM) as pmm,
            tc.tile_pool(name="psm", bufs=2, space=bass.MemorySpace.PSUM) as psm,
        ):
            # ---- prefetch the big streams first (latency-critical) ----
            # ---- weights first (small; everything downstream needs them),
            # then the big streams in use-order ----
            wi8_sb = cpool.tile([128, DC, K], FP8, tag="wi8")
            wib_sb = cpool.tile([128, DC, K], BF16, tag="wib")
            wq_sb = cpool.tile([128, DC, K], F32, tag="wq")
            bq_sb = cpool.tile([128, KC], F32, tag="bq")
            wp_sb = cpool.tile([128, KC], BF16, tag="wp")
            idf_sb = cpool.tile([128, 128], F32, tag="idf")
            idb_sb = cpool.tile([128, 128], BF16, tag="idb")
            onesc_sb = cpool.tile([128, 1], F32, tag="onesc")
            vq_sb = cpool.tile([BLOC, D], F32, tag="vqsb")
            nc.sync.dma_start(out=wi8_sb[:], in_=wi8[:])
            nc.sync.dma_start(out=wp_sb[:], in_=wpc[:])
            nc.sync.dma_start(out=wq_sb[:], in_=wq[:])
            nc.sync.dma_start(out=vq_sb[:], in_=vq[:])
            nc.sync.dma_start(out=idf_sb[:], in_=idf[:])
            nc.sync.dma_start(out=idb_sb[:], in_=idb[:])
            nc.sync.dma_start(out=onesc_sb[:], in_=onesc[:])
            nc.sync.dma_start(out=wib_sb[:], in_=wib[:])
            nc.sync.dma_start(out=bq_sb[:], in_=bqc[:])

            vit_tiles, vnat_tiles = [], []
            for b in range(BLOC):
                vit_tiles.append(
                    spool.tile([128, DC, N], FP8, tag="vit", name=f"vit{b}")
                )
                vnat_tiles.append(
                    spool.tile([128, N // 128, D], FP8, tag="vnat", name=f"vnat{b}")
                )

            def load_vit(b):
                vsrc = vit_d[b].rearrange("c p n -> p c n")
                nc.sync.dma_start(out=vit_tiles[b][:, :, 0:1024], in_=vsrc[:, :, 0:1024])
                nc.sync.dma_start(out=vit_tiles[b][:, :, 1024:N], in_=vsrc[:, :, 1024:N])

            def load_vnat(b):
                nc.sync.dma_start(out=vnat_tiles[b][:], in_=vnat_d[b])

            for b in (0, 1):
                load_vit(b)
            load_vnat(0)
            load_vit(2)
            load_vnat(1)
            load_vit(3)
            load_vnat(2)
            load_vnat(3)

            # ---- vQp (fp32, once per core, all 4 local batches) ----

            vqt_ps = psm.tile([128, DC, BLOC], F32, tag="small")
            for c in range(DC):
                nc.tensor.transpose(
                    vqt_ps[:, c, :],
                    vq_sb[:, c * 128 : (c + 1) * 128],
                    idf_sb[0:BLOC, 0:BLOC],
                )
            vqt_sb = cpool.tile([128, DC, BLOC], F32, tag="vqt")
            nc.vector.tensor_copy(vqt_sb[:], vqt_ps[:])

            # vQp^T[k, b] = sum_d Wq[d,k] vQ[b,d] + bq[k]   (K on partitions)
            vqpt_sb = cpool.tile([128, KC, BLOC], F32, tag="vqpt")
            for kc in range(KC):
                vqpt_ps = psm.tile([128, BLOC], F32, tag="small")
                for c in range(DC):
                    nc.tensor.matmul(
                        vqpt_ps[:],
                        wq_sb[:, c, kc * 128 : (kc + 1) * 128],
                        vqt_sb[:, c, :],
                        start=(c == 0),
                        stop=(c == DC - 1),
                    )
                nc.vector.tensor_scalar(
                    vqpt_sb[:, kc, :], vqpt_ps[:], bq_sb[:, kc : kc + 1], None, ALU.add
                )

            # row form vQp[b] = [1, K]  (transpose back; includes bq)
            vqpr_sb = cpool.tile([1, BLOC, K], F32, tag="vqpr")
            for b in range(BLOC):
                vqpr_ps = psm.tile([1, K], F32, tag="small")
                for kc in range(KC):
                    nc.tensor.transpose(
                        vqpr_ps[0:1, kc * 128 : (kc + 1) * 128],
                        vqpt_sb[:, kc, b : b + 1],
                        idf_sb[:],
                    )
                nc.vector.tensor_copy(vqpr_sb[:, b, :], vqpr_ps[:])

            out_sb = cpool.tile([1, BLOC, K], F32, tag="outb")

            vits, scrows = [None] * BLOC, [None] * BLOC

            def phase_scores(b):
                vit = vit_tiles[b]
                vits[b] = vit
                scrow = wpool.tile([1, N], F32, tag="scrow")
                scrows[b] = scrow
                for sp in range(N // WSUP):           # two 1024-wide supertiles
                    scps = [
                        psm.tile([1, SUP], F32, tag="small", name=f"scp{b}_{sp}_{h}")
                        for h in range(2)
                    ]
                    has = []
                    for kc in range(KC):
                        vp = pmm.tile([128, WSUP], F32, tag="vp")
                        for h in range(2):
                            n0 = sp * WSUP + h * SUP
                            for c in range(DC):
                                nc.tensor.matmul(
                                    vp[:, h * SUP : (h + 1) * SUP],
                                    wi8_sb[:, c, kc * 128 : (kc + 1) * 128],
                                    vit[:, c, n0 : n0 + SUP],
                                    start=(c == 0),
                                    stop=(c == DC - 1),
                                )
                        ha = wpool.tile([128, WSUP], BF16, tag="ha")
                        # Wi is host-scaled x16 into fp8 normal range; ACT
                        # de-scales for free: ha = lrelu(vp/16 + vqp)
                        nc.scalar.activation(
                            ha[:], vp[:], AF.Lrelu,
                            bias=vqpt_sb[:, kc, b : b + 1], scale=1.0 / 16, alpha=NEG,
                        )
                        has.append(ha)
                    for kc in range(KC):
                        for h in range(2):
                            nc.tensor.matmul(
                                scps[h][:], wp_sb[:, kc : kc + 1],
                                has[kc][:, h * SUP : (h + 1) * SUP],
                                start=(kc == 0), stop=(kc == KC - 1),
                            )
                    for h in range(2):
                        n0 = sp * WSUP + h * SUP
                        if h == 0:
                            nc.scalar.copy(scrow[0:1, n0 : n0 + SUP], scps[h][:])
                        else:
                            nc.vector.tensor_copy(scrow[0:1, n0 : n0 + SUP], scps[h][:])

            def phase_attn(b):
                vit, vnat, scrow = vits[b], vnat_tiles[b], scrows[b]
                # scores -> [16,128] -> xbar transpose -> [128,16] (partition-
                # major), then exp there. Both hops are tiny SBUF->SBUF DMAs
                # on otherwise-idle queues.
                s16 = wpool.tile([16, 128], BF16, tag="s16")
                sb16 = wpool.tile([1, N], BF16, tag="sb16")
                nc.vector.tensor_copy(sb16[:], scrow[:])
                nc.sync.dma_start(
                    out=s16[:], in_=sb16[0:1, :].rearrange("o (t p) -> o t p", p=128)
                )
                s_col = wpool.tile([128, 16], BF16, tag="scol")
                nc.sync.dma_start_transpose(out=s_col[:], in_=s16[:])

                e_col = wpool.tile([128, 16], FP8, tag="ecol")
                zp = wpool.tile([128, 1], F32, tag="zp")
                nc.scalar.activation(e_col[:], s_col[:], AF.Exp, accum_out=zp[:])
                zps = psm.tile([1, 1], F32, tag="small")
                nc.tensor.matmul(zps[:], onesc_sb[:], zp[:], start=True, stop=True)
                z_sb = wpool.tile([1, 1], F32, tag="zsb")
                nc.vector.tensor_copy(z_sb[:], zps[:])
                invz = wpool.tile([1, 1], F32, tag="invz")
                nc.vector.reciprocal(invz[:], z_sb[:])

                # u = e @ vI on the PE: 16 accumulating fp8 matmuls
                ups = psm.tile([1, D], F32, tag="small")
                for t in range(N // 128):
                    nc.tensor.matmul(
                        ups[:],
                        e_col[:, t : t + 1],
                        vnat[:, t, :],
                        start=(t == 0),
                        stop=(t == N // 128 - 1),
                    )
                u_sb = wpool.tile([1, D], BF16, tag="usb")
                nc.vector.tensor_copy(u_sb[:], ups[:])
                utp = psm.tile([128, DC, 2], BF16, tag="small")
                for c in range(DC):
                    nc.tensor.transpose(
                        utp[:, c, 0:1],
                        u_sb[0:1, c * 128 : (c + 1) * 128],
                        idb_sb[0:1, 0:1],
                    )
                ut_sb = wpool.tile([128, DC], BF16, tag="utsb")
                nc.vector.tensor_copy(ut_sb[:], utp[:, :, 0])

                # att = u @ Wi   [1, K]
                atp = psm.tile([1, K], F32, tag="small")
                for c in range(DC):
                    nc.tensor.matmul(
                        atp[:], ut_sb[:, c : c + 1], wib_sb[:, c, :],
                        start=(c == 0), stop=(c == DC - 1),
                    )
                fin = wpool.tile([1, K], F32, tag="fin")
                nc.vector.tensor_scalar(fin[:], atp[:], invz[:], None, ALU.mult)
                nc.vector.tensor_tensor(
                    out_sb[:, b, :], fin[:], vqpr_sb[:, b, :], ALU.add
                )
                if DEBUG and b == DBG_B:
                    nc.sync.dma_start(out=d_ecol[:], in_=e_col[:])
                    nc.sync.dma_start(out=d_z[:], in_=z_sb[:])
                    nc.sync.dma_start(out=d_fin[:], in_=fin[:])

            # software pipeline: scores(b+1) overlaps attention(b)
            for b in range(BLOC + 1):
                if b < BLOC:
                    phase_scores(b)
                if b >= 1:
                    phase_attn(b - 1)

            nc.sync.dma_start(out=out[:, :], in_=out_sb[0:1, :, :])

    nc.compile()
    return nc


_NC = None


def _get_nc():
    global _NC
    if _NC is None:
        _NC = build_nc()
    return _NC


def kernel(vI, vQ, Wi, Wq, bq, Wp, bp, **_unused):
    vI = np.asarray(vI, dtype=np.float32)
    vQ = np.asarray(vQ, dtype=np.float32)
    Wi = np.asarray(Wi, dtype=np.float32)
    Wq = np.asarray(Wq, dtype=np.float32)
    bq = np.asarray(bq, dtype=np.float32)
    Wp = np.asarray(Wp, dtype=np.float32)
    # bp shifts every score equally -> cancels in softmax; ignored.

    bf = ml_dtypes.bfloat16
    f8 = ml_dtypes.float8_e4m3
    # host-side: cast to fp8 and pre-transpose to [B, DC, 128, N]
    vi8 = vI.astype(f8)
    viT = np.ascontiguousarray(vi8.transpose(0, 2, 1).reshape(B, DC, 128, N))
    vnat = np.ascontiguousarray(
        vi8.reshape(B, N // 128, 128, D).transpose(0, 2, 1, 3)
    )
    wi_r = Wi.reshape(DC, 128, K).transpose(1, 0, 2)             # [128,DC,K]
    wq_h = np.ascontiguousarray(Wq.reshape(DC, 128, K).transpose(1, 0, 2))
    bq_h = np.ascontiguousarray(bq.reshape(KC, 128).T)           # [128,KC]
    wp_h = np.ascontiguousarray(Wp[:, 0].reshape(KC, 128).T).astype(bf)
    idf = np.eye(128, dtype=np.float32)
    idb = idf.astype(bf)
    onesc = np.ones((128, 1), dtype=np.float32)

    in_maps = []
    for c in range(NCORES):
        in_maps.append(
            {
                "vit": viT[c * BLOC : (c + 1) * BLOC],
                "vnat": vnat[c * BLOC : (c + 1) * BLOC],
                "vq": np.ascontiguousarray(vQ[c * BLOC : (c + 1) * BLOC]),
                "wi8": (wi_r * 16.0).astype(f8),
                "wib": wi_r.astype(bf),
                "wq": wq_h,
                "bqc": bq_h,
                "wpc": wp_h,
                "idf": idf,
                "idb": idb,
                "onesc": onesc,
            }
        )

    nc = _get_nc()
    res = run_bass_kernel_spmd(
        nc, in_maps, list(range(NCORES)),
        trace=bool(int(os.environ.get("KERNEL_TRACE", "0"))),
        tmpdir=globals().get("TRACE_TMPDIR"),
    )
    kernel.last_results = res
    return np.concatenate([res.results[c]["out"] for c in range(NCORES)], axis=0)


# revision 36
# speedup vs baseline: 1.2666x; 1.1219x over previous
"""Trainium2 Bass kernel for the attention-pooling module.

Reference math (B=32, N=2048, D=512, K=256):
    vIp   = vI @ Wi                                   [B,N,K]
    vQp   = vQ @ Wq + bq                              [B,K]
    ha    = leaky_relu(vIp + vQp[:,None,:], 0.01)     [B,N,K]
    scores= ha @ Wp[:,0] + bp                         [B,N]   (bp shift cancels in softmax)
    pi    = softmax(scores, -1)                       [B,N]
    out   = einsum("bn,bnk->bk", pi, vIp) + vQp       [B,K]

Kernel strategy (8 cores, data-parallel over B, 4 batches/core):
  - The output is vQp-dominated: vI_attn is a pi-weighted mean of ~N(0,0.58)
    rows over 2048 samples, ~40x smaller than vQp. Errors in the whole
    scores/attention path are damped accordingly, so vI streams as fp8-e4m3
    (host-cast), 1 MiB per batch; the vQp path stays fp32.
  - vI is host-transposed to [D, N] so the device streams vIT at natural-DMA
    rate and nothing on-chip transposes the bulk tensor (PE-mode transposes
    cost ~275 ns each and starve the HAM clock gate).
  - vIpT = Wi.T @ vIT in [K-on-partitions, N-on-free] layout (fp8 x fp8
    matmuls), so the vQp bias, Wp weighting and softmax map onto
    per-partition ops.
  - ha = ACT Lrelu(vIpT + vQp_k) fused, emitted as [128,1024] double-wides
    to halve ACT instruction count.
  - scores = matmul(lhsT=Wp_col, rhs=ha) accumulated over the two K chunks;
    exp without max-subtraction (|scores| < ~2) with Z via accum_out.
  - u = e @ vI on DVE via the fused affine_mul_reduce custom op against a
    GpSimd partition_broadcast of e (single pass over vIT per batch).
  - vI_attn = (u @ Wi) / Z  (exact linear refactor of pi @ vIp).
  - The scores phase (PE-bound) of batch b+1 is interleaved with the
    attention phase (DVE-bound) of batch b so the two bottleneck engines
    overlap.
"""

import os
import sys

sys.path.insert(0, "/opt/trn_rl_repo")

import numpy as np
import ml_dtypes

from concourse import bass, bacc, tile, mybir
from concourse.bass_utils import run_bass_kernel_spmd

dt = mybir.dt
F32, BF16, FP8 = dt.float32, dt.bfloat16, dt.float8e4
AF = mybir.ActivationFunctionType
ALU = mybir.AluOpType
AXF = mybir.AxisListType.X

B, N, D, K = 32, 2048, 512, 256
NCORES = 8
BLOC = B // NCORES           # 4 batches per core
SUP = 512                    # scores supertile (PSUM-bank limited)
NSUP = N // SUP              # 4
WSUP = 1024                  # ha double-wide
DC = D // 128                # 4 contraction chunks
KC = K // 128                # 2 K chunks
NEG = 0.01


def build_nc():
    nc = bacc.Bacc("TRN2", target_bir_lowering=False, debug=False)

    vit_d = nc.dram_tensor("vit", [BLOC, 128, 2, 2, N], FP8, kind="ExternalInput")
    vnat_d = nc.dram_tensor("vnat", [BLOC, 128, N // 128, D], FP8, kind="ExternalInput")
    vq = nc.dram_tensor("vq", [BLOC, D], F32, kind="ExternalInput")
    wi8 = nc.dram_tensor("wi8", [128, 2, 2, K], FP8, kind="ExternalInput")
    wib = nc.dram_tensor("wib", [128, DC, K], BF16, kind="ExternalInput")
    wq = nc.dram_tensor("wq", [128, DC, K], F32, kind="ExternalInput")
    bqc = nc.dram_tensor("bqc", [128, KC], F32, kind="ExternalInput")
    wpc = nc.dram_tensor("wpc", [128, KC], BF16, kind="ExternalInput")
    idf = nc.dram_tensor("idf", [128, 128], F32, kind="ExternalInput")
    idb = nc.dram_tensor("idb", [128, 128], BF16, kind="ExternalInput")
    onesc = nc.dram_tensor("onesc", [128, 1], F32, kind="ExternalInput")
    out = nc.dram_tensor("out", [BLOC, K], F32, kind="ExternalOutput")
    DEBUG = bool(int(os.environ.get("KERNEL_DEBUG", "0")))
    DBG_B = int(os.environ.get("KERNEL_DEBUG_B", "0"))
    if DEBUG:
        d_ecol = nc.dram_tensor("d_ecol", [128, 16], FP8, kind="ExternalOutput")
        d_z = nc.dram_tensor("d_z", [1, 1], F32, kind="ExternalOutput")
        d_fin = nc.dram_tensor("d_fin", [1, K], F32, kind="ExternalOutput")

    with tile.TileContext(nc) as tc:
        with (
            tc.tile_pool(name="const", bufs=1) as cpool,
            tc.tile_pool(name="stream", bufs=4) as spool,
            tc.tile_pool(name="work", bufs=3) as wpool,
            tc.tile_pool(name="pmm", bufs=3, space=bass.MemorySpace.PSUM) as pmm,
            tc.tile_pool(name="psm", bufs=2, space=bass.MemorySpace.PSUM) as psm,
        ):
            # ---- prefetch the big streams first (latency-critical) ----
            # ---- weights first (small; everything downstream needs them),
            # then the big streams in use-order ----
            wi8_sb = cpool.tile([128, 2, 2, K], FP8, tag="wi8")
            wib_sb = cpool.tile([128, DC, K], BF16, tag="wib")
            wq_sb = cpool.tile([128, DC, K], F32, tag="wq")
            bq_sb = cpool.tile([128, KC], F32, tag="bq")
            wp_sb = cpool.tile([128, KC], BF16, tag="wp")
            idf_sb = cpool.tile([128, 128], F32, tag="idf")
            idb_sb = cpool.tile([128, 128], BF16, tag="idb")
            onesc_sb = cpool.tile([128, 1], F32, tag="onesc")
            vq_sb = cpool.tile([BLOC, D], F32, tag="vqsb")
            nc.sync.dma_start(out=wi8_sb[:], in_=wi8[:])
            nc.sync.dma_start(out=wp_sb[:], in_=wpc[:])
            nc.sync.dma_start(out=wq_sb[:], in_=wq[:])
            nc.sync.dma_start(out=vq_sb[:], in_=vq[:])
            nc.sync.dma_start(out=idf_sb[:], in_=idf[:])
            nc.sync.dma_start(out=idb_sb[:], in_=idb[:])
            nc.sync.dma_start(out=onesc_sb[:], in_=onesc[:])
            nc.sync.dma_start(out=wib_sb[:], in_=wib[:])
            nc.sync.dma_start(out=bq_sb[:], in_=bqc[:])

            vit_tiles, vnat_tiles = [], []
            for b in range(BLOC):
                vit_tiles.append(
                    spool.tile([128, 2, 2, N], FP8, tag="vit", name=f"vit{b}")
                )
                vnat_tiles.append(
                    spool.tile([128, N // 128, D], FP8, tag="vnat", name=f"vnat{b}")
                )

            def load_vit(b):
                nc.sync.dma_start(
                    out=vit_tiles[b][:, :, :, 0:1024], in_=vit_d[b][:, :, :, 0:1024]
                )
                nc.sync.dma_start(
                    out=vit_tiles[b][:, :, :, 1024:N], in_=vit_d[b][:, :, :, 1024:N]
                )

            def load_vnat(b):
                nc.sync.dma_start(out=vnat_tiles[b][:], in_=vnat_d[b])

            for b in (0, 1):
                load_vit(b)
            load_vnat(0)
            load_vit(2)
            load_vnat(1)
            load_vit(3)
            load_vnat(2)
            load_vnat(3)

            # ---- vQp (fp32, once per core, all 4 local batches) ----

            vqt_ps = psm.tile([128, DC, BLOC], F32, tag="small")
            for c in range(DC):
                nc.tensor.transpose(
                    vqt_ps[:, c, :],
                    vq_sb[:, c * 128 : (c + 1) * 128],
                    idf_sb[0:BLOC, 0:BLOC],
                )
            vqt_sb = cpool.tile([128, DC, BLOC], F32, tag="vqt")
            nc.vector.tensor_copy(vqt_sb[:], vqt_ps[:])

            # vQp^T[k, b] = sum_d Wq[d,k] vQ[b,d] + bq[k]   (K on partitions)
            vqpt_sb = cpool.tile([128, KC, BLOC], F32, tag="vqpt")
            for kc in range(KC):
                vqpt_ps = psm.tile([128, BLOC], F32, tag="small")
                for c in range(DC):
                    nc.tensor.matmul(
                        vqpt_ps[:],
                        wq_sb[:, c, kc * 128 : (kc + 1) * 128],
                        vqt_sb[:, c, :],
                        start=(c == 0),
                        stop=(c == DC - 1),
                    )
                nc.vector.tensor_scalar(
                    vqpt_sb[:, kc, :], vqpt_ps[:], bq_sb[:, kc : kc + 1], None, ALU.add
                )

            # row form vQp[b] = [1, K]  (transpose back; includes bq)
            vqpr_sb = cpool.tile([1, BLOC, K], F32, tag="vqpr")
            for b in range(BLOC):
                vqpr_ps = psm.tile([1, K], F32, tag="small")
                for kc in range(KC):
                    nc.tensor.transpose(
                        vqpr_ps[0:1, kc * 128 : (kc + 1) * 128],
                        vqpt_sb[:, kc, b : b + 1],
                        idf_sb[:],
                    )
                nc.vector.tensor_copy(vqpr_sb[:, b, :], vqpr_ps[:])

            out_sb = cpool.tile([1, BLOC, K], F32, tag="outb")

            vits, scrows = [None] * BLOC, [None] * BLOC

            def phase_scores(b):
                vit = vit_tiles[b]
                vits[b] = vit
                scrow = wpool.tile([1, N], F32, tag="scrow")
                scrows[b] = scrow
                for sp in range(N // WSUP):           # two 1024-wide supertiles
                    scps = [
                        psm.tile([1, SUP], F32, tag="small", name=f"scp{b}_{sp}_{h}")
                        for h in range(2)
                    ]
                    has = []
                    for kc in range(KC):
                        vp = pmm.tile([128, WSUP], F32, tag="vp")
                        for h in range(2):
                            n0 = sp * WSUP + h * SUP
                            for cc in range(2):
                                nc.tensor.matmul(
                                    vp[:, h * SUP : (h + 1) * SUP],
                                    wi8_sb[:, cc, :, kc * 128 : (kc + 1) * 128],
                                    vit[:, cc, :, n0 : n0 + SUP],
                                    perf_mode=mybir.MatmulPerfMode.DoubleRow,
                                    start=(cc == 0),
                                    stop=(cc == 1),
                                )
                        ha = wpool.tile([128, WSUP], BF16, tag="ha")
                        # Wi is host-scaled x16 into fp8 normal range; ACT
                        # de-scales for free: ha = lrelu(vp/16 + vqp)
                        nc.scalar.activation(
                            ha[:], vp[:], AF.Lrelu,
                            bias=vqpt_sb[:, kc, b : b + 1], scale=1.0 / 16, alpha=NEG,
                        )
                        has.append(ha)
                    for kc in range(KC):
                        for h in range(2):
                            nc.tensor.matmul(
                                scps[h][:], wp_sb[:, kc : kc + 1],
                                has[kc][:, h * SUP : (h + 1) * SUP],
                                start=(kc == 0), stop=(kc == KC - 1),
                            )
                    for h in range(2):
                        n0 = sp * WSUP + h * SUP
                        if h == 0:
                            nc.scalar.copy(scrow[0:1, n0 : n0 + SUP], scps[h][:])
                        else:
                            nc.vector.tensor_copy(scrow[0:1, n0 : n0 + SUP], scps[h][:])

            def phase_attn(b):
                vit, vnat, scrow = vits[b], vnat_tiles[b], scrows[b]
                # scores -> [16,128] -> xbar transpose -> [128,16] (partition-
                # major), then exp there. Both hops are tiny SBUF->SBUF DMAs
                # on otherwise-idle queues.
                s16 = wpool.tile([16, 128], BF16, tag="s16")
                sb16 = wpool.tile([1, N], BF16, tag="sb16")
                nc.vector.tensor_copy(sb16[:], scrow[:])
                nc.sync.dma_start(
                    out=s16[:], in_=sb16[0:1, :].rearrange("o (t p) -> o t p", p=128)
                )
                s_col = wpool.tile([128, 16], BF16, tag="scol")
                nc.sync.dma_start_transpose(out=s_col[:], in_=s16[:])

                e_col = wpool.tile([128, 16], FP8, tag="ecol")
                zp = wpool.tile([128, 1], F32, tag="zp")
                nc.scalar.activation(e_col[:], s_col[:], AF.Exp, accum_out=zp[:])
                zps = psm.tile([1, 1], F32, tag="small")
                nc.tensor.matmul(zps[:], onesc_sb[:], zp[:], start=True, stop=True)
                z_sb = wpool.tile([1, 1], F32, tag="zsb")
                nc.vector.tensor_copy(z_sb[:], zps[:])
                invz = wpool.tile([1, 1], F32, tag="invz")
                nc.vector.reciprocal(invz[:], z_sb[:])

                # u = e @ vI on the PE: 16 accumulating fp8 matmuls
                ups = psm.tile([1, D], F32, tag="small")
                for t in range(N // 128):
                    nc.tensor.matmul(
                        ups[:],
                        e_col[:, t : t + 1],
                        vnat[:, t, :],
                        start=(t == 0),
                        stop=(t == N // 128 - 1),
                    )
                u_sb = wpool.tile([1, D], BF16, tag="usb")
                nc.vector.tensor_copy(u_sb[:], ups[:])
                utp = psm.tile([128, DC, 2], BF16, tag="small")
                for c in range(DC):
                    nc.tensor.transpose(
                        utp[:, c, 0:1],
                        u_sb[0:1, c * 128 : (c + 1) * 128],
                        idb_sb[0:1, 0:1],
                    )
                ut_sb = wpool.tile([128, DC], BF16, tag="utsb")
                nc.vector.tensor_copy(ut_sb[:], utp[:, :, 0])

                # att = u @ Wi   [1, K]
                atp = psm.tile([1, K], F32, tag="small")
                for c in range(DC):
                    nc.tensor.matmul(
                        atp[:], ut_sb[:, c : c + 1], wib_sb[:, c, :],
                        start=(c == 0), stop=(c == DC - 1),
                    )
                fin = wpool.tile([1, K], F32, tag="fin")
                nc.vector.tensor_scalar(fin[:], atp[:], invz[:], None, ALU.mult)
                nc.vector.tensor_tensor(
                    out_sb[:, b, :], fin[:], vqpr_sb[:, b, :], ALU.add
                )
                if DEBUG and b == DBG_B:
                    nc.sync.dma_start(out=d_ecol[:], in_=e_col[:])
                    nc.sync.dma_start(out=d_z[:], in_=z_sb[:])
                    nc.sync.dma_start(out=d_fin[:], in_=fin[:])

            # software pipeline: scores(b+1) overlaps attention(b)
            for b in range(BLOC + 1):
                if b < BLOC:
                    phase_scores(b)
                if b >= 1:
                    phase_attn(b - 1)

            nc.sync.dma_start(out=out[:, :], in_=out_sb[0:1, :, :])

    nc.compile()
    return nc


_NC = None


def _get_nc():
    global _NC
    if _NC is None:
        _NC = build_nc()
    return _NC


def kernel(vI, vQ, Wi, Wq, bq, Wp, bp, **_unused):
    vI = np.asarray(vI, dtype=np.float32)
    vQ = np.asarray(vQ, dtype=np.float32)
    Wi = np.asarray(Wi, dtype=np.float32)
    Wq = np.asarray(Wq, dtype=np.float32)
    bq = np.asarray(bq, dtype=np.float32)
    Wp = np.asarray(Wp, dtype=np.float32)
    # bp shifts every score equally -> cancels in softmax; ignored.

    bf = ml_dtypes.bfloat16
    f8 = ml_dtypes.float8_e4m3
    # host-side: cast to fp8 and pre-transpose to [B, DC, 128, N]
    vi8 = vI.astype(f8)
    # DoubleRow layout: d = cc*256 + i*128 + p  ->  [B, p, cc, i, N]
    viT = np.ascontiguousarray(
        vi8.transpose(0, 2, 1).reshape(B, 2, 2, 128, N).transpose(0, 3, 1, 2, 4)
    )
    vnat = np.ascontiguousarray(
        vi8.reshape(B, N // 128, 128, D).transpose(0, 2, 1, 3)
    )
    wi_r = Wi.reshape(DC, 128, K).transpose(1, 0, 2)             # [128,DC,K]
    wi8_dr = np.ascontiguousarray(
        (Wi * 16.0).reshape(2, 2, 128, K).transpose(2, 0, 1, 3)
    ).astype(f8)                                                  # [128,cc,i,K]
    wq_h = np.ascontiguousarray(Wq.reshape(DC, 128, K).transpose(1, 0, 2))
    bq_h = np.ascontiguousarray(bq.reshape(KC, 128).T)           # [128,KC]
    wp_h = np.ascontiguousarray(Wp[:, 0].reshape(KC, 128).T).astype(bf)
    idf = np.eye(128, dtype=np.float32)
    idb = idf.astype(bf)
    onesc = np.ones((128, 1), dtype=np.float32)

    in_maps = []
    for c in range(NCORES):
        in_maps.append(
            {
                "vit": viT[c * BLOC : (c + 1) * BLOC],
                "vnat": vnat[c * BLOC : (c + 1) * BLOC],
                "vq": np.ascontiguousarray(vQ[c * BLOC : (c + 1) * BLOC]),
                "wi8": wi8_dr,
                "wib": wi_r.astype(bf),
                "wq": wq_h,
                "bqc": bq_h,
                "wpc": wp_h,
                "idf": idf,
                "idb": idb,
                "onesc": onesc,
            }
        )

    nc = _get_nc()
    res = run_bass_kernel_spmd(
        nc, in_maps, list(range(NCORES)),
        trace=bool(int(os.environ.get("KERNEL_TRACE", "0"))),
        tmpdir=globals().get("TRACE_TMPDIR"),
    )
    kernel.last_results = res
    return np.concatenate([res.results[c]["out"] for c in range(NCORES)], axis=0)


# revision 39
# speedup vs baseline: 1.4184x; 1.1198x over previous
"""Trainium2 Bass kernel for the attention-pooling module.

Reference math (B=32, N=2048, D=512, K=256):
    vIp   = vI @ Wi                                   [B,N,K]
    vQp   = vQ @ Wq + bq                              [B,K]
    ha    = leaky_relu(vIp + vQp[:,None,:], 0.01)     [B,N,K]
    scores= ha @ Wp[:,0] + bp                         [B,N]   (bp shift cancels in softmax)
    pi    = softmax(scores, -1)                       [B,N]
    out   = einsum("bn,bnk->bk", pi, vIp) + vQp       [B,K]

Kernel strategy (8 cores, data-parallel over B, 4 batches/core):
  - The output is vQp-dominated: vI_attn is a pi-weighted mean of ~N(0,0.58)
    rows over 2048 samples, ~40x smaller than vQp. Errors in the whole
    scores/attention path are damped accordingly, so vI streams as fp8-e4m3
    (host-cast), 1 MiB per batch; the vQp path stays fp32.
  - vI is host-transposed to [D, N] so the device streams vIT at natural-DMA
    rate and nothing on-chip transposes the bulk tensor (PE-mode transposes
    cost ~275 ns each and starve the HAM clock gate).
  - vIpT = Wi.T @ vIT in [K-on-partitions, N-on-free] layout (fp8 x fp8
    matmuls), so the vQp bias, Wp weighting and softmax map onto
    per-partition ops.
  - ha = ACT Lrelu(vIpT + vQp_k) fused, emitted as [128,1024] double-wides
    to halve ACT instruction count.
  - scores = matmul(lhsT=Wp_col, rhs=ha) accumulated over the two K chunks;
    exp without max-subtraction (|scores| < ~2) with Z via accum_out.
  - u = e @ vI on DVE via the fused affine_mul_reduce custom op against a
    GpSimd partition_broadcast of e (single pass over vIT per batch).
  - vI_attn = (u @ Wi) / Z  (exact linear refactor of pi @ vIp).
  - The scores phase (PE-bound) of batch b+1 is interleaved with the
    attention phase (DVE-bound) of batch b so the two bottleneck engines
    overlap.
"""

import os
import sys

sys.path.insert(0, "/opt/trn_rl_repo")

import numpy as np
import ml_dtypes

from concourse import bass, bacc, tile, mybir
from concourse.bass_utils import run_bass_kernel_spmd

dt = mybir.dt
F32, BF16, FP8 = dt.float32, dt.bfloat16, dt.float8e4
AF = mybir.ActivationFunctionType
ALU = mybir.AluOpType
AXF = mybir.AxisListType.X

B, N, D, K = 32, 2048, 512, 256
NCORES = 8
BLOC = B // NCORES           # 4 batches per core
SUP = 512                    # scores supertile (PSUM-bank limited)
NSUP = N // SUP              # 4
WSUP = 1024                  # ha double-wide
DC = D // 128                # 4 contraction chunks
KC = K // 128                # 2 K chunks
NEG = 0.01


def build_nc():
    nc = bacc.Bacc("TRN2", target_bir_lowering=False, debug=False)

    vit_d = nc.dram_tensor("vit", [BLOC, 128, 2, 2, N], FP8, kind="ExternalInput")
    vnat_d = nc.dram_tensor("vnat", [BLOC, 128, N // 128, D], FP8, kind="ExternalInput")
    vq = nc.dram_tensor("vq", [BLOC, D], F32, kind="ExternalInput")
    wi8 = nc.dram_tensor("wi8", [128, 2, 2, K], FP8, kind="ExternalInput")
    wib = nc.dram_tensor("wib", [128, DC, K], BF16, kind="ExternalInput")
    wq = nc.dram_tensor("wq", [128, DC, K], F32, kind="ExternalInput")
    bqc = nc.dram_tensor("bqc", [128, KC], F32, kind="ExternalInput")
    wpc = nc.dram_tensor("wpc", [128, KC], BF16, kind="ExternalInput")
    idf = nc.dram_tensor("idf", [128, 128], F32, kind="ExternalInput")
    idb = nc.dram_tensor("idb", [128, 128], BF16, kind="ExternalInput")
    onesc = nc.dram_tensor("onesc", [128, 1], F32, kind="ExternalInput")
    out = nc.dram_tensor("out", [BLOC, K], F32, kind="ExternalOutput")
    DEBUG = bool(int(os.environ.get("KERNEL_DEBUG", "0")))
    DBG_B = int(os.environ.get("KERNEL_DEBUG_B", "0"))
    if DEBUG:
        d_ecol = nc.dram_tensor("d_ecol", [128, 16], FP8, kind="ExternalOutput")
        d_z = nc.dram_tensor("d_z", [1, 1], F32, kind="ExternalOutput")
        d_fin = nc.dram_tensor("d_fin", [1, K], F32, kind="ExternalOutput")

    with tile.TileContext(nc) as tc:
        with (
            tc.tile_pool(name="const", bufs=1) as cpool,
            tc.tile_pool(name="stream", bufs=4) as spool,
            tc.tile_pool(name="work", bufs=3) as wpool,
            tc.tile_pool(name="pmm", bufs=3, space=bass.MemorySpace.PSUM) as pmm,
            tc.tile_pool(name="psm", bufs=2, space=bass.MemorySpace.PSUM) as psm,
        ):
            # ---- prefetch the big streams first (latency-critical) ----
            # ---- weights first (small; everything downstream needs them),
            # then the big streams in use-order ----
            wi8_sb = cpool.tile([128, 2, 2, K], FP8, tag="wi8")
            wib_sb = cpool.tile([128, DC, K], BF16, tag="wib")
            wq_sb = cpool.tile([128, DC, K], F32, tag="wq")
            bq_sb = cpool.tile([128, KC], F32, tag="bq")
            wp_sb = cpool.tile([128, KC], BF16, tag="wp")
            idf_sb = cpool.tile([128, 128], F32, tag="idf")
            idb_sb = cpool.tile([128, 128], BF16, tag="idb")
            onesc_sb = cpool.tile([128, 1], F32, tag="onesc")
            vq_sb = cpool.tile([BLOC, D], F32, tag="vqsb")
            nc.sync.dma_start(out=wi8_sb[:], in_=wi8[:])
            nc.sync.dma_start(out=wp_sb[:], in_=wpc[:])
            nc.sync.dma_start(out=wq_sb[:], in_=wq[:])
            nc.sync.dma_start(out=vq_sb[:], in_=vq[:])
            nc.sync.dma_start(out=idf_sb[:], in_=idf[:])
            nc.sync.dma_start(out=idb_sb[:], in_=idb[:])
            nc.sync.dma_start(out=onesc_sb[:], in_=onesc[:])
            nc.sync.dma_start(out=wib_sb[:], in_=wib[:])
            nc.sync.dma_start(out=bq_sb[:], in_=bqc[:])

            vit_tiles, vnat_tiles = [], []
            for b in range(BLOC):
                vit_tiles.append(
                    spool.tile([128, 2, 2, N], FP8, tag="vit", name=f"vit{b}")
                )
                vnat_tiles.append(
                    spool.tile([128, N // 128, D], FP8, tag="vnat", name=f"vnat{b}")
                )

            def load_vit(b):
                nc.sync.dma_start(
                    out=vit_tiles[b][:, :, :, 0:1024], in_=vit_d[b][:, :, :, 0:1024]
                )
                nc.sync.dma_start(
                    out=vit_tiles[b][:, :, :, 1024:N], in_=vit_d[b][:, :, :, 1024:N]
                )

            def load_vnat(b):
                nc.sync.dma_start(out=vnat_tiles[b][:], in_=vnat_d[b])

            nc.sync.dma_start(
                out=vit_tiles[0][:, :, :, 0:512], in_=vit_d[0][:, :, :, 0:512]
            )
            nc.sync.dma_start(
                out=vit_tiles[0][:, :, :, 512:N], in_=vit_d[0][:, :, :, 512:N]
            )
            load_vit(1)
            load_vnat(0)
            load_vit(2)
            load_vnat(1)
            load_vit(3)
            load_vnat(2)
            load_vnat(3)

            # ---- vQp (fp32, once per core, all 4 local batches) ----

            vqt_ps = psm.tile([128, DC, BLOC], F32, tag="small")
            for c in range(DC):
                nc.tensor.transpose(
                    vqt_ps[:, c, :],
                    vq_sb[:, c * 128 : (c + 1) * 128],
                    idf_sb[0:BLOC, 0:BLOC],
                )
            vqt_sb = cpool.tile([128, DC, BLOC], F32, tag="vqt")
            nc.vector.tensor_copy(vqt_sb[:], vqt_ps[:])

            # vQp^T[k, b] = sum_d Wq[d,k] vQ[b,d] + bq[k]   (K on partitions)
            vqpt_sb = cpool.tile([128, KC, BLOC], F32, tag="vqpt")
            for kc in range(KC):
                vqpt_ps = psm.tile([128, BLOC], F32, tag="small")
                for c in range(DC):
                    nc.tensor.matmul(
                        vqpt_ps[:],
                        wq_sb[:, c, kc * 128 : (kc + 1) * 128],
                        vqt_sb[:, c, :],
                        start=(c == 0),
                        stop=(c == DC - 1),
                    )
                nc.vector.tensor_scalar(
                    vqpt_sb[:, kc, :], vqpt_ps[:], bq_sb[:, kc : kc + 1], None, ALU.add
                )

            # row form vQp[b] = [1, K]  (transpose back; includes bq)
            vqpr_sb = cpool.tile([1, BLOC, K], F32, tag="vqpr")
            for b in range(BLOC):
                vqpr_ps = psm.tile([1, K], F32, tag="small")
                for kc in range(KC):
                    nc.tensor.transpose(
                        vqpr_ps[0:1, kc * 128 : (kc + 1) * 128],
                        vqpt_sb[:, kc, b : b + 1],
                        idf_sb[:],
                    )
                nc.vector.tensor_copy(vqpr_sb[:, b, :], vqpr_ps[:])

            out_sb = cpool.tile([1, BLOC, K], F32, tag="outb")

            vits, scrows = [None] * BLOC, [None] * BLOC

            def phase_scores(b):
                vit = vit_tiles[b]
                vits[b] = vit
                scrow = wpool.tile([1, N], BF16, tag="scrow")
                scrows[b] = scrow
                for sp in range(N // WSUP):           # two 1024-wide supertiles
                    scps = [
                        psm.tile([1, SUP], F32, tag="small", name=f"scp{b}_{sp}_{h}")
                        for h in range(2)
                    ]
                    has = []
                    for kc in range(KC):
                        vp = pmm.tile([128, WSUP], F32, tag="vp")
                        for h in range(2):
                            n0 = sp * WSUP + h * SUP
                            for cc in range(2):
                                nc.tensor.matmul(
                                    vp[:, h * SUP : (h + 1) * SUP],
                                    wi8_sb[:, cc, :, kc * 128 : (kc + 1) * 128],
                                    vit[:, cc, :, n0 : n0 + SUP],
                                    perf_mode=mybir.MatmulPerfMode.DoubleRow,
                                    start=(cc == 0),
                                    stop=(cc == 1),
                                )
                        ha = wpool.tile([128, WSUP], BF16, tag="ha")
                        # Wi is host-scaled x16 into fp8 normal range; ACT
                        # de-scales for free: ha = lrelu(vp/16 + vqp)
                        nc.scalar.activation(
                            ha[:], vp[:], AF.Lrelu,
                            bias=vqpt_sb[:, kc, b : b + 1], scale=1.0 / 16, alpha=NEG,
                        )
                        has.append(ha)
                    for kc in range(KC):
                        for h in range(2):
                            nc.tensor.matmul(
                                scps[h][:], wp_sb[:, kc : kc + 1],
                                has[kc][:, h * SUP : (h + 1) * SUP],
                                start=(kc == 0), stop=(kc == KC - 1),
                            )
                    for h in range(2):
                        n0 = sp * WSUP + h * SUP
                        if h == 0:
                            nc.scalar.copy(scrow[0:1, n0 : n0 + SUP], scps[h][:])
                        else:
                            nc.vector.tensor_copy(scrow[0:1, n0 : n0 + SUP], scps[h][:])

            def phase_attn(b):
                vit, vnat, scrow = vits[b], vnat_tiles[b], scrows[b]
                # scores -> [16,128] -> xbar transpose -> [128,16] (partition-
                # major), then exp there. Both hops are tiny SBUF->SBUF DMAs
                # on otherwise-idle queues.
                s16 = wpool.tile([16, 128], BF16, tag="s16")
                nc.sync.dma_start(
                    out=s16[:], in_=scrow[0:1, :].rearrange("o (t p) -> o t p", p=128)
                )
                s_col = wpool.tile([128, 16], BF16, tag="scol")
                nc.sync.dma_start_transpose(out=s_col[:], in_=s16[:])

                # [128, 2, 16]: pair partner at +16B so the DoubleRow
                # lhsT AP satisfies the 16B-step ISA constraint
                e_col = wpool.tile([128, 2, 16], FP8, tag="ecol")
                zp = wpool.tile([128, 1], F32, tag="zp")
                nc.scalar.activation(
                    e_col[:].rearrange("p i j -> p j i")[:, 0:8, :],
                    s_col[:].rearrange("p (j i) -> p j i", i=2),
                    AF.Exp, accum_out=zp[:],
                )
                zps = psm.tile([1, 1], F32, tag="small")
                nc.tensor.matmul(zps[:], onesc_sb[:], zp[:], start=True, stop=True)
                z_sb = wpool.tile([1, 1], F32, tag="zsb")
                nc.vector.tensor_copy(z_sb[:], zps[:])
                invz = wpool.tile([1, 1], F32, tag="invz")
                nc.vector.reciprocal(invz[:], z_sb[:])

                # u = e @ vI on the PE: 16 accumulating fp8 matmuls
                ups = psm.tile([1, D], F32, tag="small")
                NT = N // 128
                for t in range(0, NT, 2):
                    nc.tensor.matmul(
                        ups[:],
                        e_col[:, :, t // 2 : t // 2 + 1],  # pair stride 16B
                        vnat[:, t : t + 2, :],
                        perf_mode=mybir.MatmulPerfMode.DoubleRow,
                        start=(t == 0),
                        stop=(t == NT - 2),
                    )
                u_sb = wpool.tile([1, D], BF16, tag="usb")
                nc.vector.tensor_copy(u_sb[:], ups[:])
                utp = psm.tile([128, DC, 2], BF16, tag="small")
                for c in range(DC):
                    nc.tensor.transpose(
                        utp[:, c, 0:1],
                        u_sb[0:1, c * 128 : (c + 1) * 128],
                        idb_sb[0:1, 0:1],
                    )
                ut_sb = wpool.tile([128, DC], BF16, tag="utsb")
                nc.vector.tensor_copy(ut_sb[:], utp[:, :, 0])

                # att = u @ Wi   [1, K]
                atp = psm.tile([1, K], F32, tag="small")
                for c in range(DC):
                    nc.tensor.matmul(
                        atp[:], ut_sb[:, c : c + 1], wib_sb[:, c, :],
                        start=(c == 0), stop=(c == DC - 1),
                    )
                fin = wpool.tile([1, K], F32, tag="fin")
                nc.vector.tensor_scalar(fin[:], atp[:], invz[:], None, ALU.mult)
                nc.vector.tensor_tensor(
                    out_sb[:, b, :], fin[:], vqpr_sb[:, b, :], ALU.add
                )
                if DEBUG and b == DBG_B:
                    nc.sync.dma_start(out=d_ecol[:, 0:8], in_=e_col[:, 0, 0:8])
                    nc.sync.dma_start(out=d_z[:], in_=z_sb[:])
                    nc.sync.dma_start(out=d_fin[:], in_=fin[:])

            # software pipeline: scores(b+1) overlaps attention(b)
            for b in range(BLOC + 1):
                if b < BLOC:
                    phase_scores(b)
                if b >= 1:
                    phase_attn(b - 1)

            nc.sync.dma_start(out=out[:, :], in_=out_sb[0:1, :, :])

    nc.compile()
    return nc


_NC = None


def _get_nc():
    global _NC
    if _NC is None:
        _NC = build_nc()
    return _NC


def kernel(vI, vQ, Wi, Wq, bq, Wp, bp, **_unused):
    vI = np.asarray(vI, dtype=np.float32)
    vQ = np.asarray(vQ, dtype=np.float32)
    Wi = np.asarray(Wi, dtype=np.float32)
    Wq = np.asarray(Wq, dtype=np.float32)
    bq = np.asarray(bq, dtype=np.float32)
    Wp = np.asarray(Wp, dtype=np.float32)
    # bp shifts every score equally -> cancels in softmax; ignored.

    bf = ml_dtypes.bfloat16
    f8 = ml_dtypes.float8_e4m3
    # host-side: cast to fp8 and pre-transpose to [B, DC, 128, N]
    vi8 = vI.astype(f8)
    # DoubleRow layout: d = cc*256 + i*128 + p  ->  [B, p, cc, i, N]
    viT = np.ascontiguousarray(
        vi8.transpose(0, 2, 1).reshape(B, 2, 2, 128, N).transpose(0, 3, 1, 2, 4)
    )
    vnat = np.ascontiguousarray(
        vi8.reshape(B, N // 128, 128, D).transpose(0, 2, 1, 3)
    )
    wi_r = Wi.reshape(DC, 128, K).transpose(1, 0, 2)             # [128,DC,K]
    wi8_dr = np.ascontiguousarray(
        (Wi * 16.0).reshape(2, 2, 128, K).transpose(2, 0, 1, 3)
    ).astype(f8)                                                  # [128,cc,i,K]
    wq_h = np.ascontiguousarray(Wq.reshape(DC, 128, K).transpose(1, 0, 2))
    bq_h = np.ascontiguousarray(bq.reshape(KC, 128).T)           # [128,KC]
    wp_h = np.ascontiguousarray(Wp[:, 0].reshape(KC, 128).T).astype(bf)
    idf = np.eye(128, dtype=np.float32)
    idb = idf.astype(bf)
    onesc = np.ones((128, 1), dtype=np.float32)

    in_maps = []
    for c in range(NCORES):
        in_maps.append(
            {
                "vit": viT[c * BLOC : (c + 1) * BLOC],
                "vnat": vnat[c * BLOC : (c + 1) * BLOC],
                "vq": np.ascontiguousarray(vQ[c * BLOC : (c + 1) * BLOC]),
                "wi8": wi8_dr,
                "wib": wi_r.astype(bf),
                "wq": wq_h,
                "bqc": bq_h,
                "wpc": wp_h,
                "idf": idf,
                "idb": idb,
                "onesc": onesc,
            }
        )

    nc = _get_nc()
    res = run_bass_kernel_spmd(
        nc, in_maps, list(range(NCORES)),
        trace=bool(int(os.environ.get("KERNEL_TRACE", "0"))),
        tmpdir=globals().get("TRACE_TMPDIR"),
    )
    kernel.last_results = res
    return np.concatenate([res.results[c]["out"] for c in range(NCORES)], axis=0)


# revision 45
# speedup vs baseline: 1.4960x; 1.0547x over previous
"""Trainium2 Bass kernel for the attention-pooling module.

Reference math (B=32, N=2048, D=512, K=256):
    vIp   = vI @ Wi                                   [B,N,K]
    vQp   = vQ @ Wq + bq                              [B,K]
    ha    = leaky_relu(vIp + vQp[:,None,:], 0.01)     [B,N,K]
    scores= ha @ Wp[:,0] + bp                         [B,N]   (bp shift cancels in softmax)
    pi    = softmax(scores, -1)                       [B,N]
    out   = einsum("bn,bnk->bk", pi, vIp) + vQp       [B,K]

Kernel strategy (8 cores, data-parallel over B, 4 batches/core):
  - The output is vQp-dominated: vI_attn is a pi-weighted mean of ~N(0,0.58)
    rows over 2048 samples, ~40x smaller than vQp. Errors in the whole
    scores/attention path are damped accordingly, so vI streams as fp8-e4m3
    (host-cast), 1 MiB per batch; the vQp path stays fp32.
  - vI is host-transposed to [D, N] so the device streams vIT at natural-DMA
    rate and nothing on-chip transposes the bulk tensor (PE-mode transposes
    cost ~275 ns each and starve the HAM clock gate).
  - vIpT = Wi.T @ vIT in [K-on-partitions, N-on-free] layout (fp8 x fp8
    matmuls), so the vQp bias, Wp weighting and softmax map onto
    per-partition ops.
  - ha = ACT Lrelu(vIpT + vQp_k) fused, emitted as [128,1024] double-wides
    to halve ACT instruction count.
  - scores = matmul(lhsT=Wp_col, rhs=ha) accumulated over the two K chunks;
    exp without max-subtraction (|scores| < ~2) with Z via accum_out.
  - u = e @ vI on DVE via the fused affine_mul_reduce custom op against a
    GpSimd partition_broadcast of e (single pass over vIT per batch).
  - vI_attn = (u @ Wi) / Z  (exact linear refactor of pi @ vIp).
  - The scores phase (PE-bound) of batch b+1 is interleaved with the
    attention phase (DVE-bound) of batch b so the two bottleneck engines
    overlap.
"""

import os
import sys

sys.path.insert(0, "/opt/trn_rl_repo")

import numpy as np
import ml_dtypes

from concourse import bass, bacc, tile, mybir
from concourse.bass_utils import run_bass_kernel_spmd

dt = mybir.dt
F32, BF16, FP8 = dt.float32, dt.bfloat16, dt.float8e4
AF = mybir.ActivationFunctionType
ALU = mybir.AluOpType
AXF = mybir.AxisListType.X

B, N, D, K = 32, 2048, 512, 256
NCORES = 8
BLOC = B // NCORES           # 4 batches per core
SUP = 512                    # scores supertile (PSUM-bank limited)
NSUP = N // SUP              # 4
WSUP = 1024                  # ha double-wide
DC = D // 128                # 4 contraction chunks
KC = K // 128                # 2 K chunks
NEG = 0.01


def build_nc():
    nc = bacc.Bacc("TRN2", target_bir_lowering=False, debug=False)

    vit_d = nc.dram_tensor("vit", [BLOC, 128, 2, 2, N], FP8, kind="ExternalInput")
    vnat_d = nc.dram_tensor("vnat", [BLOC, 128, N // 128, D], FP8, kind="ExternalInput")
    wi8 = nc.dram_tensor("wi8", [128, 2, 2, K], FP8, kind="ExternalInput")
    pk32 = nc.dram_tensor("pk32", [128, 1171], F32, kind="ExternalInput")
    pk16 = nc.dram_tensor("pk16", [128, 1184], BF16, kind="ExternalInput")
    out = nc.dram_tensor("out", [BLOC, K], F32, kind="ExternalOutput")
    DEBUG = bool(int(os.environ.get("KERNEL_DEBUG", "0")))
    DBG_B = int(os.environ.get("KERNEL_DEBUG_B", "0"))
    if DEBUG:
        d_ecol = nc.dram_tensor("d_ecol", [128, 16], FP8, kind="ExternalOutput")
        d_z = nc.dram_tensor("d_z", [1, 1], F32, kind="ExternalOutput")
        d_fin = nc.dram_tensor("d_fin", [1, K], F32, kind="ExternalOutput")

    with tile.TileContext(nc) as tc:
        with (
            tc.tile_pool(name="const", bufs=1) as cpool,
            tc.tile_pool(name="stream", bufs=4) as spool,
            tc.tile_pool(name="work", bufs=3) as wpool,
            tc.tile_pool(name="pmm", bufs=3, space=bass.MemorySpace.PSUM) as pmm,
            tc.tile_pool(name="psm", bufs=2, space=bass.MemorySpace.PSUM) as psm,
        ):
            # ---- weights in 3 packed DMAs (DMA-issue on Sync costs ~1us
            # each; fewer, bigger transfers start compute sooner) ----
            wi8_sb = cpool.tile([128, 2, 2, K], FP8, tag="wi8")
            pk32_sb = cpool.tile([128, 1171], F32, tag="pk32")
            pk16_sb = cpool.tile([128, 1184], BF16, tag="pk16")
            nc.sync.dma_start(out=wi8_sb[:], in_=wi8[:])
            nc.sync.dma_start(out=pk16_sb[:], in_=pk16[:])
            nc.sync.dma_start(out=pk32_sb[:], in_=pk32[:])
            wq_sb = pk32_sb[:, 0:1024].rearrange("p (c k) -> p c k", c=DC)
            idf_sb = pk32_sb[:, 1024:1152]
            vqt_sb = pk32_sb[:, 1152:1168].rearrange("p (c b) -> p c b", c=DC)
            bq_sb = pk32_sb[:, 1168:1170]
            onesc_sb = pk32_sb[:, 1170:1171]
            wib_sb = pk16_sb[:, 0:1024].rearrange("p (c k) -> p c k", c=DC)
            idb_sb = pk16_sb[:, 1024:1152]
            wp_dr16 = pk16_sb[:, 1152:1184].rearrange("p (i j) -> p i j", i=2)
            wp8 = cpool.tile([128, 2, 16], FP8, tag="wp8")
            nc.vector.tensor_copy(wp8[:], wp_dr16[:])

            vit_tiles, vnat_tiles = [], []
            for b in range(BLOC):
                vit_tiles.append(
                    spool.tile([128, 2, 2, N], FP8, tag="vit", name=f"vit{b}")
                )
                vnat_tiles.append(
                    spool.tile([128, N // 128, D], FP8, tag="vnat", name=f"vnat{b}")
                )

            def load_vit(b):
                nc.sync.dma_start(
                    out=vit_tiles[b][:, :, :, 0:1024], in_=vit_d[b][:, :, :, 0:1024]
                )
                nc.sync.dma_start(
                    out=vit_tiles[b][:, :, :, 1024:N], in_=vit_d[b][:, :, :, 1024:N]
                )

            def load_vnat(b):
                nc.sync.dma_start(out=vnat_tiles[b][:], in_=vnat_d[b])

            nc.sync.dma_start(
                out=vit_tiles[0][:, :, :, 0:512], in_=vit_d[0][:, :, :, 0:512]
            )
            nc.sync.dma_start(
                out=vit_tiles[0][:, :, :, 512:N], in_=vit_d[0][:, :, :, 512:N]
            )
            load_vit(1)
            load_vnat(0)
            load_vit(2)
            load_vnat(1)
            load_vit(3)
            load_vnat(2)
            load_vnat(3)

            # ---- vQp (fp32, once per core, all 4 local batches) ----

            # vQp^T[k, b] = sum_d Wq[d,k] vQ[b,d] + bq[k]   (K on partitions)
            vqpt_sb = cpool.tile([128, KC, BLOC], F32, tag="vqpt")
            for kc in range(KC):
                vqpt_ps = psm.tile([128, BLOC], F32, tag="small")
                for c in range(DC):
                    nc.tensor.matmul(
                        vqpt_ps[:],
                        wq_sb[:, c, kc * 128 : (kc + 1) * 128],
                        vqt_sb[:, c, :],
                        start=(c == 0),
                        stop=(c == DC - 1),
                    )
                nc.vector.tensor_scalar(
                    vqpt_sb[:, kc, :], vqpt_ps[:], bq_sb[:, kc : kc + 1], None, ALU.add
                )

            # row form vQp[b] = [1, K]  (transpose back; includes bq)
            vqpr_sb = cpool.tile([1, BLOC, K], F32, tag="vqpr")
            for b in range(BLOC):
                vqpr_ps = psm.tile([1, K], F32, tag="small")
                for kc in range(KC):
                    nc.tensor.transpose(
                        vqpr_ps[0:1, kc * 128 : (kc + 1) * 128],
                        vqpt_sb[:, kc, b : b + 1],
                        idf_sb[:],
                    )
                nc.vector.tensor_copy(vqpr_sb[:, b, :], vqpr_ps[:])

            out_sb = cpool.tile([1, BLOC, K], F32, tag="outb")

            vits, scrows = [None] * BLOC, [None] * BLOC

            def phase_scores(b):
                vit = vit_tiles[b]
                vits[b] = vit
                scrow = wpool.tile([1, N], BF16, tag="scrow")
                scrows[b] = scrow
                for sp in range(N // WSUP):           # two 1024-wide supertiles
                    scps = [
                        psm.tile([1, SUP], F32, tag="small", name=f"scp{b}_{sp}_{h}")
                        for h in range(2)
                    ]
                    ha = wpool.tile([128, KC, WSUP], FP8, tag="ha")
                    for kc in range(KC):
                        vp = pmm.tile([128, WSUP], F32, tag="vp")
                        for h in range(2):
                            n0 = sp * WSUP + h * SUP
                            for cc in range(2):
                                nc.tensor.matmul(
                                    vp[:, h * SUP : (h + 1) * SUP],
                                    wi8_sb[:, cc, :, kc * 128 : (kc + 1) * 128],
                                    vit[:, cc, :, n0 : n0 + SUP],
                                    perf_mode=mybir.MatmulPerfMode.DoubleRow,
                                    start=(cc == 0),
                                    stop=(cc == 1),
                                )
                        # Wi is host-scaled x16 into fp8 normal range; ACT
                        # de-scales for free: ha = lrelu(vp/16 + vqp)
                        nc.scalar.activation(
                            ha[:, kc, :], vp[:], AF.Lrelu,
                            bias=vqpt_sb[:, kc, b : b + 1], scale=1.0 / 16, alpha=NEG,
                        )
                    for h in range(2):
                        nc.tensor.matmul(
                            scps[h][:], wp8[:, :, 0:1],
                            ha[:, :, h * SUP : (h + 1) * SUP],
                            perf_mode=mybir.MatmulPerfMode.DoubleRow,
                            start=True, stop=True,
                        )
                    for h in range(2):
                        n0 = sp * WSUP + h * SUP
                        nc.vector.tensor_copy(scrow[0:1, n0 : n0 + SUP], scps[h][:])

            def phase_attn(b):
                vit, vnat, scrow = vits[b], vnat_tiles[b], scrows[b]
                # scores -> [16,128] -> xbar transpose -> [128,16] (partition-
                # major), then exp there. Both hops are tiny SBUF->SBUF DMAs
                # on otherwise-idle queues.
                s16 = wpool.tile([16, 128], BF16, tag="s16")
                nc.sync.dma_start(
                    out=s16[:], in_=scrow[0:1, :].rearrange("o (t p) -> o t p", p=128)
                )
                s_col = wpool.tile([128, 16], BF16, tag="scol")
                nc.sync.dma_start_transpose(out=s_col[:], in_=s16[:])

                # [128, 2, 16]: pair partner at +16B so the DoubleRow
                # lhsT AP satisfies the 16B-step ISA constraint
                e_col = wpool.tile([128, 2, 16], FP8, tag="ecol")
                zp = wpool.tile([128, 1], F32, tag="zp")
                # Wp is host-scaled x8 (fp8 range); exp de-scales for free
                nc.scalar.activation(
                    e_col[:].rearrange("p i j -> p j i")[:, 0:8, :],
                    s_col[:].rearrange("p (j i) -> p j i", i=2),
                    AF.Exp, scale=1.0 / 8, accum_out=zp[:],
                )
                zps = psm.tile([1, 1], F32, tag="small")
                nc.tensor.matmul(zps[:], onesc_sb[:], zp[:], start=True, stop=True)
                z_sb = wpool.tile([1, 1], F32, tag="zsb")
                nc.vector.tensor_copy(z_sb[:], zps[:])
                invz = wpool.tile([1, 1], F32, tag="invz")
                nc.vector.reciprocal(invz[:], z_sb[:])

                # u = e @ vI on the PE: 16 accumulating fp8 matmuls
                ups = psm.tile([1, D], F32, tag="small")
                NT = N // 128
                for t in range(0, NT, 2):
                    nc.tensor.matmul(
                        ups[:],
                        e_col[:, :, t // 2 : t // 2 + 1],  # pair stride 16B
                        vnat[:, t : t + 2, :],
                        perf_mode=mybir.MatmulPerfMode.DoubleRow,
                        start=(t == 0),
                        stop=(t == NT - 2),
                    )
                u_sb = wpool.tile([1, D], BF16, tag="usb")
                nc.vector.tensor_copy(u_sb[:], ups[:])
                utp = psm.tile([128, DC, 2], BF16, tag="small")
                for c in range(DC):
                    nc.tensor.transpose(
                        utp[:, c, 0:1],
                        u_sb[0:1, c * 128 : (c + 1) * 128],
                        idb_sb[0:1, 0:1],
                    )
                ut_sb = wpool.tile([128, DC], BF16, tag="utsb")
                nc.vector.tensor_copy(ut_sb[:], utp[:, :, 0])

                # att = u @ Wi   [1, K]
                atp = psm.tile([1, K], F32, tag="small")
                for c in range(DC):
                    nc.tensor.matmul(
                        atp[:], ut_sb[:, c : c + 1], wib_sb[:, c, :],
                        start=(c == 0), stop=(c == DC - 1),
                    )
                fin = wpool.tile([1, K], F32, tag="fin")
                nc.vector.tensor_scalar(fin[:], atp[:], invz[:], None, ALU.mult)
                nc.vector.tensor_tensor(
                    out_sb[:, b, :], fin[:], vqpr_sb[:, b, :], ALU.add
                )
                if DEBUG and b == DBG_B:
                    nc.sync.dma_start(out=d_ecol[:, 0:8], in_=e_col[:, 0, 0:8])
                    nc.sync.dma_start(out=d_z[:], in_=z_sb[:])
                    nc.sync.dma_start(out=d_fin[:], in_=fin[:])

            # software pipeline: scores(b+1) overlaps attention(b)
            for b in range(BLOC + 1):
                if b < BLOC:
                    phase_scores(b)
                if b >= 1:
                    phase_attn(b - 1)

            nc.sync.dma_start(out=out[:, :], in_=out_sb[0:1, :, :])

    nc.compile()
    return nc


_NC = None


def _get_nc():
    global _NC
    if _NC is None:
        _NC = build_nc()
    return _NC


def kernel(vI, vQ, Wi, Wq, bq, Wp, bp, **_unused):
    vI = np.asarray(vI, dtype=np.float32)
    vQ = np.asarray(vQ, dtype=np.float32)
    Wi = np.asarray(Wi, dtype=np.float32)
    Wq = np.asarray(Wq, dtype=np.float32)
    bq = np.asarray(bq, dtype=np.float32)
    Wp = np.asarray(Wp, dtype=np.float32)
    # bp shifts every score equally -> cancels in softmax; ignored.

    bf = ml_dtypes.bfloat16
    f8 = ml_dtypes.float8_e4m3
    # host-side: cast to fp8 and pre-transpose to [B, DC, 128, N]
    vi8 = vI.astype(f8)
    # DoubleRow layout: d = cc*256 + i*128 + p  ->  [B, p, cc, i, N]
    viT = np.ascontiguousarray(
        vi8.transpose(0, 2, 1).reshape(B, 2, 2, 128, N).transpose(0, 3, 1, 2, 4)
    )
    vnat = np.ascontiguousarray(
        vi8.reshape(B, N // 128, 128, D).transpose(0, 2, 1, 3)
    )
    wi_r = Wi.reshape(DC, 128, K).transpose(1, 0, 2)             # [128,DC,K]
    wi8_dr = np.ascontiguousarray(
        (Wi * 16.0).reshape(2, 2, 128, K).transpose(2, 0, 1, 3)
    ).astype(f8)                                                  # [128,cc,i,K]
    wq_h = Wq.reshape(DC, 128, K).transpose(1, 0, 2).reshape(128, DC * K)
    bq_h = bq.reshape(KC, 128).T                                 # [128,KC]
    wp_h = Wp[:, 0].reshape(KC, 128).T                           # [128,KC]
    idf = np.eye(128, dtype=np.float32)
    onesc = np.ones((128, 1), dtype=np.float32)

    # pk16: wib(1024) | idb(128) | wp_dr(2x16, wp in col j=0)
    wp_pad = np.zeros((128, 2, 16), np.float32)
    wp_pad[:, :, 0] = wp_h * 8.0
    pk16 = np.concatenate(
        [wi_r.reshape(128, DC * K), idf, wp_pad.reshape(128, 32)], axis=1
    ).astype(bf)

    # pk32 per-core: wq(1024) | idf(128) | vqt(16) | bqc(2) | onesc(1)
    def pk32_for(core):
        vqc = vQ[core * BLOC : (core + 1) * BLOC]                # [BLOC, D]
        vqt = vqc.T.reshape(DC, 128, BLOC).transpose(1, 0, 2)    # [128,DC,BLOC]
        return np.ascontiguousarray(
            np.concatenate(
                [wq_h, idf, vqt.reshape(128, DC * BLOC), bq_h, onesc], axis=1
            )
        ).astype(np.float32)

    in_maps = []
    for c in range(NCORES):
        in_maps.append(
            {
                "vit": viT[c * BLOC : (c + 1) * BLOC],
                "vnat": vnat[c * BLOC : (c + 1) * BLOC],
                "wi8": wi8_dr,
                "pk16": pk16,
                "pk32": pk32_for(c),
            }
        )

    nc = _get_nc()
    res = run_bass_kernel_spmd(
        nc, in_maps, list(range(NCORES)),
        trace=bool(int(os.environ.get("KERNEL_TRACE", "0"))),
        tmpdir=globals().get("TRACE_TMPDIR"),
    )
    kernel.last_results = res
    return np.concatenate([res.results[c]["out"] for c in range(NCORES)], axis=0)
